# revision 1
# baseline (speedup 1.0000x reference)
"""Trainium2 Bass kernel for a 2-layer bidirectional GRU + linear head.

Problem: B=64, S=4096, D_IN=7, H=128, PyTorch gate order (r, z, n).
Sharding: data-parallel over batch across 8 NeuronCores (8 rows each).

Per-core design (all layouts keep H=128 on the SBUF partition axis):
  - The sequence is processed in chunks of C=64 steps. For each chunk the
    input-gate projections gx = W_ih @ x (+ biases) for the r,z gates of both
    directions are computed by bulk matmuls directly into a 4-bank PSUM tile
    [128, 4, C*8]; the per-step recurrent matmuls (W_hh @ h) then accumulate
    onto their 8-column slice (start=False), so sigmoid reads (xr+hr, xz+hz)
    straight out of PSUM with zero staging ops.
  - The n-gate projections go to an SBUF ring (xn must not receive W_hh@h
    before the r* multiply); b_hh_n is staged into a small PSUM tile with a
    rank-2 matmul, and W_hh_n@h accumulates there.
  - Both directions are packed into the free dim of every elementwise op
    (columns 0:8 forward, 8:16 backward); the backward direction consumes a
    host-reversed copy of x so all its tensors are in scan order ("u" order),
    and the reversal is applied via negative-stride APs when layer 1 / the
    head need time-aligned pairs.
  - The hidden-state ring [128, C, 16] doubles as the output buffer: the
    final h' add of each step writes the ring slot, which the next step's
    matmuls read as rhs and which is DMA'd to DRAM per chunk.
"""

import numpy as np

import concourse.bass as bass
import concourse.tile as tile
from concourse import bacc, mybir
from concourse.bass import ds

F32 = mybir.dt.float32
AF = mybir.ActivationFunctionType

H = 128
DIN = 7
B = 64
NCORES = 8
BL = B // NCORES  # batch rows per core


DEBUG_DUMPS = False
STEP_MODE = "full"   # "full" | "nochain" (steps read hstate, no serial dep) | "nostep"
# timing ablations: "act_copy" (sigmoid/tanh -> Copy), "no_rzmm" (drop 4 rz
# matmuls), "no_nmm" (drop psn matmuls), "no_upd" (drop d/zd/h' DVE ops),
# "no_rn" (drop rn/arg DVE ops)
ABLATIONS = set()
L1_FWD_ONLY = False  # debug: layer1 reads h0 chunks forward (wrong results)
LAYER_BARRIER = False
USE_HINTS = True
SPLIT_RZ = False     # four 1-bank PSUM tiles + per-gate sigmoid instead of one 4-bank tile
SKIP_L1 = False      # emit only layer 0; head reads h0f/h0b
SKIP_HEAD = False    # skip the head phase (out left zero)


def build_program(S=4096, C=64, n_cores=NCORES):
    """Build the per-core Bass program. Returns (nc, bout_placeholder_used)."""
    NCH = S // C
    W = C * BL  # chunk columns (= matmul moving-dim), 512 for C=64
    nc = bacc.Bacc("TRN2", target_bir_lowering=False, debug=False)
    dbg = {}
    if DEBUG_DUMPS:
        dbg["rz"] = nc.dram_tensor("dbg_rz", [H, 4, BL], F32, kind="ExternalOutput").ap()
        dbg["psn"] = nc.dram_tensor("dbg_psn", [H, 2 * BL], F32, kind="ExternalOutput").ap()
        dbg["arg"] = nc.dram_tensor("dbg_arg", [H, 2 * BL], F32, kind="ExternalOutput").ap()
        dbg["gxn"] = nc.dram_tensor("dbg_gxn", [H, 2 * BL], F32, kind="ExternalOutput").ap()

    # ---- DRAM I/O ----
    xf = nc.dram_tensor("xf", [DIN + 1, S * BL], F32, kind="ExternalInput").ap()
    xr = nc.dram_tensor("xr", [DIN + 1, S * BL], F32, kind="ExternalInput").ap()
    whhT = nc.dram_tensor("whhT", [12, H, H], F32, kind="ExternalInput").ap()
    wih0T = nc.dram_tensor("wih0T", [2, DIN + 1, 3 * H], F32, kind="ExternalInput").ap()
    wih1T = nc.dram_tensor("wih1T", [2, 2, H, 3 * H], F32, kind="ExternalInput").ap()
    bias1 = nc.dram_tensor("bias1", [2, 3 * H], F32, kind="ExternalInput").ap()
    bhhn2 = nc.dram_tensor("bhhn2", [2, 2, H], F32, kind="ExternalInput").ap()
    sel2 = nc.dram_tensor("sel2", [2, 2 * BL], F32, kind="ExternalInput").ap()
    woutp = nc.dram_tensor("woutp", [H, 2], F32, kind="ExternalInput").ap()
    boutp = nc.dram_tensor("boutp", [1, 1], F32, kind="ExternalInput").ap()
    ones = nc.dram_tensor("ones", [1, W], F32, kind="ExternalInput").ap()
    out = nc.dram_tensor("out", [S, BL], F32, kind="ExternalOutput").ap()
    out_flat = out.rearrange("s b -> (s b)")

    with tile.TileContext(nc) as tc:
        from contextlib import ExitStack

        stack = ExitStack()
        consts = stack.enter_context(tc.tile_pool(name="consts", bufs=1))
        dramp = stack.enter_context(tc.tile_pool(name="dramp", bufs=1, space="DRAM"))

        # ---- persistent SBUF constants ----
        whh_sb = consts.tile([H, 12 * H], F32)  # (l,d,g) blocks of 128 cols
        for k in range(12):
            nc.sync.dma_start(whh_sb[:, k * H:(k + 1) * H], whhT[k])
        wih0_sb = consts.tile([DIN + 1, 2 * 3 * H], F32)
        for d in range(2):
            nc.sync.dma_start(wih0_sb[:, d * 3 * H:(d + 1) * 3 * H], wih0T[d])
        wih1_sb = consts.tile([H, 4 * 3 * H], F32)  # (d,k) blocks of 384 cols
        for d in range(2):
            for k in range(2):
                c0 = (d * 2 + k) * 3 * H
                nc.sync.dma_start(wih1_sb[:, c0:c0 + 3 * H], wih1T[d, k])
        bias1_sb = consts.tile([1, 2 * 3 * H], F32)
        nc.sync.dma_start(bias1_sb[:], bias1.rearrange("d m -> (d m)"))
        bhhn_sb = consts.tile([2, 2 * H], F32)  # [dir_row, layer*128+col]
        for l in range(2):
            nc.sync.dma_start(bhhn_sb[:, l * H:(l + 1) * H], bhhn2[l])
        sel2_sb = consts.tile([2, 2 * BL], F32)
        nc.sync.dma_start(sel2_sb[:], sel2[:])
        wout_sb = consts.tile([H, 2], F32)
        nc.sync.dma_start(wout_sb[:], woutp[:])
        bout_sb = consts.tile([1, 1], F32)
        nc.sync.dma_start(bout_sb[:], boutp[:])
        ones_sb = consts.tile([1, W], F32)
        nc.sync.dma_start(ones_sb[:], ones[:])
        hstate = consts.tile([H, 2 * BL], F32)

        # ---- internal DRAM: layer outputs (backward dir in scan order) ----
        h0f = nc.dram_tensor("h0f", [H, S, BL], F32, kind="Internal").ap()
        h0b = nc.dram_tensor("h0b", [H, S, BL], F32, kind="Internal").ap()
        h1f = nc.dram_tensor("h1f", [H, S, BL], F32, kind="Internal").ap()
        h1b = nc.dram_tensor("h1b", [H, S, BL], F32, kind="Internal").ap()

        def whh(l, d, g):
            k = (l * 2 + d) * 3 + g
            return whh_sb[:, k * H:(k + 1) * H]

        rec = ExitStack()
        rhsp = rec.enter_context(tc.tile_pool(name="rhsp", bufs=2))
        gxnp = rec.enter_context(tc.tile_pool(name="gxnp", bufs=2))
        ringp = rec.enter_context(tc.tile_pool(name="ringp", bufs=2))
        stepp = rec.enter_context(tc.tile_pool(name="stepp", bufs=3))
        psp = rec.enter_context(tc.tile_pool(name="psp", bufs=1, space="PSUM"))
        psnjp = rec.enter_context(tc.tile_pool(name="psnjp", bufs=2, space="PSUM"))
        psnp = rec.enter_context(tc.tile_pool(name="psnp", bufs=2, space="PSUM"))

        def emit_step(l, j, ring, gxn, rz_ps, rz_tiles=None):
            if j == 0 or STEP_MODE == "nochain":
                hf, hb = hstate[:, 0:BL], hstate[:, BL:2 * BL]
            else:
                hf, hb = ring[:, j - 1, 0:BL], ring[:, j - 1, BL:2 * BL]
            js = slice(j * BL, (j + 1) * BL)

            def rzd(sl):
                if rz_tiles is not None:
                    return rz_tiles[sl][:, js]
                return rz_ps[:, sl, js]
            SIG = AF.Copy if "act_copy" in ABLATIONS else AF.Sigmoid
            TANH = AF.Copy if "act_copy" in ABLATIONS else AF.Tanh
            # hn = b_hh_n + W_hh_n @ h  (both dirs) in small psum
            psn = psnp.tile([H, 2 * BL], F32, tag="psn")
            nc.tensor.matmul(psn[:], bhhn_sb[:, l * H:(l + 1) * H], sel2_sb[:],
                             start=True, stop=False, skip_group_check=True)
            if "no_nmm" not in ABLATIONS:
                nc.tensor.matmul(psn[:, 0:BL], whh(l, 0, 2), hf,
                                 start=False, stop=False, skip_group_check=True)
                nc.tensor.matmul(psn[:, BL:2 * BL], whh(l, 1, 2), hb,
                                 start=False, stop=True, skip_group_check=True)
            # r,z gates accumulate onto the prefilled gx slices
            if "no_rzmm" not in ABLATIONS:
                nc.tensor.matmul(rzd(0), whh(l, 0, 0), hf,
                                 start=False, stop=False, skip_group_check=True)
                nc.tensor.matmul(rzd(1), whh(l, 1, 0), hb,
                                 start=False, stop=False, skip_group_check=True)
                nc.tensor.matmul(rzd(2), whh(l, 0, 1), hf,
                                 start=False, stop=False, skip_group_check=True)
                nc.tensor.matmul(rzd(3), whh(l, 1, 1), hb,
                                 start=False, stop=(j == C - 1), skip_group_check=True)
            rz = stepp.tile([H, 4, BL], F32, tag="rz")
            if rz_tiles is not None:
                for k in range(4):
                    nc.scalar.activation(rz[:, k, :], rzd(k), SIG)
            else:
                nc.scalar.activation(rz[:], rz_ps[:, :, js], SIG)
            if DEBUG_DUMPS and l == 0 and j == 0:
                psn_sb = stepp.tile([H, 2 * BL], F32, tag="psndbg")
                nc.vector.tensor_copy(psn_sb[:], psn[:])
                nc.sync.dma_start(dbg["psn"], psn_sb[:])
                nc.sync.dma_start(dbg["rz"], rz[:])
                nc.sync.dma_start(dbg["gxn"], gxn[:, :, js])
            if "no_rn" not in ABLATIONS:
                rn = stepp.tile([H, 2 * BL], F32, tag="rn")
                nc.vector.tensor_mul(rn[:], rz[:, 0:2, :], psn[:])
                arg = stepp.tile([H, 2 * BL], F32, tag="arg")
                nc.vector.tensor_add(arg[:], rn[:], gxn[:, :, js])
                tanh_in = arg
            else:
                tanh_in = None
            if DEBUG_DUMPS and l == 0 and j == 0:
                nc.sync.dma_start(dbg["arg"], arg[:])
            n_t = stepp.tile([H, 2 * BL], F32, tag="n")
            if tanh_in is not None:
                nc.scalar.activation(n_t[:], tanh_in[:], TANH)
            else:
                nc.scalar.activation(n_t[:], gxn[:, :, js], TANH)
            if "no_upd" not in ABLATIONS:
                d_t = stepp.tile([H, 2 * BL], F32, tag="d")
                h_prev = (hstate[:, :] if (j == 0 or STEP_MODE == "nochain")
                          else ring[:, j - 1, :])
                nc.vector.tensor_sub(d_t[:], h_prev, n_t[:])
                zd = stepp.tile([H, 2 * BL], F32, tag="zd")
                nc.vector.tensor_mul(zd[:], rz[:, 2:4, :], d_t[:])
                nc.vector.tensor_add(ring[:, j, :], n_t[:], zd[:])
            else:
                nc.vector.tensor_copy(ring[:, j, :], n_t[:])

        def emit_layer(l):
            nc.vector.memset(hstate[:], 0.0)
            h_f_dst, h_b_dst = (h0f, h0b) if l == 0 else (h1f, h1b)
            hints = (mybir.EngineType.PE, mybir.EngineType.DVE) if USE_HINTS else ()
            with tc.For_i(0, NCH, 1, name=f"layer{l}", hint_engines=hints) as i:
                if SPLIT_RZ:
                    rz_tiles = [psp.tile([H, W], F32, tag=f"rzps{k}", name=f"rzps{k}")
                                for k in range(4)]
                    rz_ps = None
                else:
                    rz_ps = psp.tile([H, 4, W], F32, tag="rzps")
                gxn = gxnp.tile([H, 2, W], F32, tag="gxn")
                ring = ringp.tile([H, C, 2 * BL], F32, tag="ring")
                # start=True clears the whole 2KB PSUM bank, so it may only be
                # used by the first matmul that touches each bank of rz_ps.
                seen_banks = set()

                def rz_start(sl):
                    bank = sl if SPLIT_RZ else sl * W // 512
                    if bank in seen_banks:
                        return False
                    seen_banks.add(bank)
                    return True

                def rz_full(sl):
                    if SPLIT_RZ:
                        return rz_tiles[sl][:, :]
                    return rz_ps[:, sl, :]

                if l == 0:
                    xf_ch = rhsp.tile([DIN + 1, W], F32, tag="xf")
                    nc.sync.dma_start(xf_ch[:], xf[:, ds(i * W, W)])
                    xr_ch = rhsp.tile([DIN + 1, W], F32, tag="xr")
                    nc.sync.dma_start(xr_ch[:], xr[:, ds(i * W, W)])
                    srcs = (xf_ch, xr_ch)
                    for dd, src in enumerate(srcs):
                        for g in range(2):  # r, z -> psum
                            nc.tensor.matmul(
                                rz_full(2 * g + dd),
                                wih0_sb[:, dd * 3 * H + g * H: dd * 3 * H + (g + 1) * H],
                                src[:], start=rz_start(2 * g + dd), stop=False,
                                skip_group_check=True)
                        nj = psnjp.tile([H, W], F32, tag="nj")
                        nc.tensor.matmul(
                            nj[:],
                            wih0_sb[:, dd * 3 * H + 2 * H: dd * 3 * H + 3 * H],
                            src[:], start=True, stop=True, skip_group_check=True)
                        # psum -> sbuf n-ring, split across DVE and ACT
                        hw = W // 2
                        nc.vector.tensor_copy(gxn[:, dd, 0:hw], nj[:, 0:hw])
                        nc.scalar.copy(gxn[:, dd, hw:W], nj[:, hw:W])
                else:
                    # Reversed reads: negative-stride dynamic DRAM APs hang the
                    # device, so read the mirrored chunk forward and reverse on
                    # the SBUF side of the DMA (static negative stride).
                    h0f_v, h0b_v = h0f[:], h0b[:]
                    mir = ds((NCH - 1 - i) * C, C)
                    ff = rhsp.tile([H, C, BL], F32, tag="ff")
                    nc.sync.dma_start(ff[:], h0f_v[:, ds(i * C, C), :])
                    brv = rhsp.tile([H, C, BL], F32, tag="brv")
                    nc.sync.dma_start(brv[:, ::-1, :], h0b_v[:, mir, :])
                    frv = rhsp.tile([H, C, BL], F32, tag="frv")
                    nc.sync.dma_start(frv[:, ::-1, :], h0f_v[:, mir, :])
                    bb = rhsp.tile([H, C, BL], F32, tag="bb")
                    nc.sync.dma_start(bb[:], h0b_v[:, ds(i * C, C), :])
                    for dd, (rA, rB) in enumerate(((ff, brv), (frv, bb))):
                        base = dd * 2 * 3 * H
                        for g in range(2):
                            dst = rz_full(2 * g + dd)
                            nc.tensor.matmul(dst, wih1_sb[:, base + g * H: base + (g + 1) * H],
                                             rA[:], start=rz_start(2 * g + dd), stop=False,
                                             skip_group_check=True)
                            nc.tensor.matmul(dst, wih1_sb[:, base + 3 * H + g * H: base + 3 * H + (g + 1) * H],
                                             rB[:], start=False, stop=False, skip_group_check=True)
                            nc.tensor.matmul(dst, bias1_sb[:, dd * 3 * H + g * H: dd * 3 * H + (g + 1) * H],
                                             ones_sb[:], start=False, stop=False, skip_group_check=True)
                        nj = psnjp.tile([H, W], F32, tag="nj")
                        nc.tensor.matmul(nj[:], wih1_sb[:, base + 2 * H: base + 3 * H],
                                         rA[:], start=True, stop=False, skip_group_check=True)
                        nc.tensor.matmul(nj[:], wih1_sb[:, base + 3 * H + 2 * H: base + 3 * H + 3 * H],
                                         rB[:], start=False, stop=False, skip_group_check=True)
                        nc.tensor.matmul(nj[:], bias1_sb[:, dd * 3 * H + 2 * H: dd * 3 * H + 3 * H],
                                         ones_sb[:], start=False, stop=True, skip_group_check=True)
                        hw = W // 2
                        nc.vector.tensor_copy(gxn[:, dd, 0:hw], nj[:, 0:hw])
                        nc.scalar.copy(gxn[:, dd, hw:W], nj[:, hw:W])

                if STEP_MODE != "nostep":
                    for j in range(C):
                        emit_step(l, j, ring, gxn, rz_ps,
                                  rz_tiles if SPLIT_RZ else None)
                else:
                    nc.vector.memset(ring[:], 0.0)

                nc.vector.tensor_copy(hstate[:], ring[:, C - 1, :])
                nc.sync.dma_start(h_f_dst[:][:, ds(i * C, C), :], ring[:, :, 0:BL])
                nc.sync.dma_start(h_b_dst[:][:, ds(i * C, C), :], ring[:, :, BL:2 * BL])

        emit_layer(0)
        if LAYER_BARRIER:
            tc.strict_bb_all_engine_barrier()
        if not SKIP_L1:
            emit_layer(1)
        else:
            h1f, h1b = h0f, h0b
        rec.close()

        # ---- head: logits = wout_f . f1[s] + wout_b . b1[s] + bout ----
        if not SKIP_HEAD:
            with tc.tile_pool(name="headp", bufs=3) as hp, \
                 tc.tile_pool(name="headps", bufs=2, space="PSUM") as hps:
                for k in range(NCH):
                    fch = hp.tile([H, W], F32, tag="fch")
                    nc.sync.dma_start(fch[:], h1f[:][:, k * C:(k + 1) * C, :])
                    bch = hp.tile([H, C, BL], F32, tag="bch")
                    mk = NCH - 1 - k
                    nc.sync.dma_start(bch[:, ::-1, :], h1b[:][:, mk * C:(mk + 1) * C, :])
                    pso = hps.tile([1, W], F32, tag="pso")
                    nc.tensor.matmul(pso[:], wout_sb[:, 0:1], fch[:],
                                     start=True, stop=False, skip_group_check=True)
                    nc.tensor.matmul(pso[:], wout_sb[:, 1:2], bch[:],
                                     start=False, stop=True, skip_group_check=True)
                    osb = hp.tile([1, W], F32, tag="osb")
                    nc.scalar.activation(osb[:], pso[:], AF.Identity,
                                         bias=bout_sb[0:1, 0:1])
                    nc.sync.dma_start(out_flat[k * W:(k + 1) * W], osb[:])
        stack.close()

    nc.compile()
    return nc


_PROGRAM_CACHE = {}


def _get_program(S=4096, C=64):
    key = (S, C)
    if key not in _PROGRAM_CACHE:
        _PROGRAM_CACHE[key] = build_program(S, C)
    return _PROGRAM_CACHE[key]


def _pack_host_inputs(inputs, S=4096, C=64):
    """Build the per-core input maps from the full problem inputs."""
    W = C * BL
    x = np.asarray(inputs["x"], np.float32)

    def gT(w, g):  # transposed gate block: [in, H]
        return np.ascontiguousarray(np.asarray(w, np.float32)[g * H:(g + 1) * H].T)

    whhT = np.stack([
        gT(inputs[f"whh{l}{d}"], g)
        for l in range(2) for d in "fb" for g in range(3)
    ])  # [12,H,H]

    wih0T = np.zeros((2, DIN + 1, 3 * H), np.float32)
    bhhn2 = np.zeros((2, 2, H), np.float32)
    for di, d in enumerate("fb"):
        wih = np.asarray(inputs[f"wih0{d}"], np.float32)  # [3H, DIN]
        bih = np.asarray(inputs[f"bih0{d}"], np.float32)
        bhh = np.asarray(inputs[f"bhh0{d}"], np.float32)
        wih0T[di, :DIN] = wih.T
        for g in range(3):
            bias = bih[g * H:(g + 1) * H].copy()
            if g < 2:
                bias += bhh[g * H:(g + 1) * H]
            wih0T[di, DIN, g * H:(g + 1) * H] = bias
        bhhn2[0, di] = bhh[2 * H:]

    wih1T = np.zeros((2, 2, H, 3 * H), np.float32)
    bias1 = np.zeros((2, 3 * H), np.float32)
    for di, d in enumerate("fb"):
        wih = np.asarray(inputs[f"wih1{d}"], np.float32)  # [3H, 2H]
        bih = np.asarray(inputs[f"bih1{d}"], np.float32)
        bhh = np.asarray(inputs[f"bhh1{d}"], np.float32)
        for k in range(2):
            for g in range(3):
                wih1T[di, k, :, g * H:(g + 1) * H] = wih[g * H:(g + 1) * H, k * H:(k + 1) * H].T
        for g in range(3):
            bias = bih[g * H:(g + 1) * H].copy()
            if g < 2:
                bias += bhh[g * H:(g + 1) * H]
            bias1[di, g * H:(g + 1) * H] = bias
        bhhn2[1, di] = bhh[2 * H:]

    sel2 = np.zeros((2, 2 * BL), np.float32)
    sel2[0, :BL] = 1.0
    sel2[1, BL:] = 1.0
    woutp = np.zeros((H, 2), np.float32)
    wout = np.asarray(inputs["wout"], np.float32)
    woutp[:, 0] = wout[0, :H]
    woutp[:, 1] = wout[0, H:]
    boutp = np.asarray(inputs["bout"], np.float32).reshape(1, 1)
    ones = np.ones((1, W), np.float32)

    shared = dict(whhT=whhT, wih0T=wih0T, wih1T=wih1T, bias1=bias1,
                  bhhn2=bhhn2, sel2=sel2, woutp=woutp, boutp=boutp, ones=ones)

    in_maps = []
    for c in range(NCORES):
        xc = x[c * BL:(c + 1) * BL]  # [BL, S, DIN]
        arr = np.ones((DIN + 1, S, BL), np.float32)
        arr[:DIN] = xc.transpose(2, 1, 0)
        xfm = np.ascontiguousarray(arr.reshape(DIN + 1, S * BL))
        xrm = np.ascontiguousarray(arr[:, ::-1, :].reshape(DIN + 1, S * BL))
        in_maps.append(dict(shared, xf=xfm, xr=xrm))
    return in_maps


def kernel(**inputs) -> np.ndarray:
    from concourse import bass_utils
    S, C = 4096, 64
    nc = _get_program(S, C)
    in_maps = _pack_host_inputs(inputs, S, C)
    res = bass_utils.run_bass_kernel_spmd(nc, in_maps, core_ids=list(range(NCORES)))
    outs = [r["out"] for r in res.results]  # each [S, BL]
    return np.concatenate([o.T for o in outs], axis=0).astype(np.float32)



# revision 2
# speedup vs baseline: 1.3669x; 1.3669x over previous
"""Trainium2 Bass kernel for a 2-layer bidirectional GRU + linear head.

Problem: B=64, S=4096, D_IN=7, H=128, PyTorch gate order (r, z, n).
Sharding: data-parallel over batch across 8 NeuronCores (8 rows each).

Per-core design (all layouts keep H=128 on the SBUF partition axis):
  - The sequence is processed in chunks of C=64 steps. For each chunk the
    input-gate projections gx = W_ih @ x (+ biases) for the r,z gates of both
    directions are computed by bulk matmuls directly into a 4-bank PSUM tile
    [128, 4, C*8]; the per-step recurrent matmuls (W_hh @ h) then accumulate
    onto their 8-column slice (start=False), so sigmoid reads (xr+hr, xz+hz)
    straight out of PSUM with zero staging ops.
  - The n-gate projections go to an SBUF ring (xn must not receive W_hh@h
    before the r* multiply); b_hh_n is staged into a small PSUM tile with a
    rank-2 matmul, and W_hh_n@h accumulates there.
  - Both directions are packed into the free dim of every elementwise op
    (columns 0:8 forward, 8:16 backward); the backward direction consumes a
    host-reversed copy of x so all its tensors are in scan order ("u" order),
    and the reversal is applied via negative-stride APs when layer 1 / the
    head need time-aligned pairs.
  - The hidden-state ring [128, C, 16] doubles as the output buffer: the
    final h' add of each step writes the ring slot, which the next step's
    matmuls read as rhs and which is DMA'd to DRAM per chunk.
  - All matmul operands are bf16 (PSUM accumulation and every elementwise op
    stay fp32): fp32 matmuls are decomposed by HW into TWO ldweights+matmul
    passes (fp32_mode LOW/HIGH) and disable Fast Weight Load, roughly
    doubling PE occupancy, and fp32 PE activity triggers the HAM power
    throttle (50% duty cycle). bf16 halves the PE instruction stream.
"""

import numpy as np
import ml_dtypes

import concourse.bass as bass
import concourse.tile as tile
from concourse import bacc, mybir
from concourse.bass import ds

F32 = mybir.dt.float32
BF16 = mybir.dt.bfloat16
NP_BF16 = ml_dtypes.bfloat16
AF = mybir.ActivationFunctionType

H = 128
DIN = 7
B = 64
NCORES = 8
BL = B // NCORES  # batch rows per core


STEP_MODE = "full"   # "full" | "nochain" (steps read hstate, no serial dep) | "nostep"
ABLATIONS = set()
USE_HINTS = True
SKIP_L1 = False      # emit only layer 0; head reads h0f/h0b
SKIP_HEAD = False    # skip the head phase (out left zero)


def build_program(S=4096, C=64, n_cores=NCORES):
    """Build the per-core Bass program."""
    NCH = S // C
    W = C * BL  # chunk columns (= matmul moving-dim), 512 for C=64
    nc = bacc.Bacc("TRN2", target_bir_lowering=False, debug=False)

    # ---- DRAM I/O ----
    xf = nc.dram_tensor("xf", [DIN + 1, S * BL], BF16, kind="ExternalInput").ap()
    xr = nc.dram_tensor("xr", [DIN + 1, S * BL], BF16, kind="ExternalInput").ap()
    whhT = nc.dram_tensor("whhT", [12, H, H], BF16, kind="ExternalInput").ap()
    wih0T = nc.dram_tensor("wih0T", [2, DIN + 1, 3 * H], BF16, kind="ExternalInput").ap()
    wih1T = nc.dram_tensor("wih1T", [2, 2, H, 3 * H], BF16, kind="ExternalInput").ap()
    bias1 = nc.dram_tensor("bias1", [2, 3 * H], BF16, kind="ExternalInput").ap()
    bhhn2 = nc.dram_tensor("bhhn2", [2, 2, H], BF16, kind="ExternalInput").ap()
    sel2 = nc.dram_tensor("sel2", [2, 2 * BL], BF16, kind="ExternalInput").ap()
    woutp = nc.dram_tensor("woutp", [H, 2], BF16, kind="ExternalInput").ap()
    boutp = nc.dram_tensor("boutp", [1, 1], F32, kind="ExternalInput").ap()
    ones = nc.dram_tensor("ones", [1, W], BF16, kind="ExternalInput").ap()
    out = nc.dram_tensor("out", [S, BL], F32, kind="ExternalOutput").ap()
    out_flat = out.rearrange("s b -> (s b)")

    with tile.TileContext(nc) as tc:
        from contextlib import ExitStack

        stack = ExitStack()
        consts = stack.enter_context(tc.tile_pool(name="consts", bufs=1))

        # ---- persistent SBUF constants (all bf16 matmul operands) ----
        whh_sb = consts.tile([H, 12 * H], BF16)  # (l,d,g) blocks of 128 cols
        for k in range(12):
            nc.sync.dma_start(whh_sb[:, k * H:(k + 1) * H], whhT[k])
        wih0_sb = consts.tile([DIN + 1, 2 * 3 * H], BF16)
        for d in range(2):
            nc.sync.dma_start(wih0_sb[:, d * 3 * H:(d + 1) * 3 * H], wih0T[d])
        wih1_sb = consts.tile([H, 4 * 3 * H], BF16)  # (d,k) blocks of 384 cols
        for d in range(2):
            for k in range(2):
                c0 = (d * 2 + k) * 3 * H
                nc.sync.dma_start(wih1_sb[:, c0:c0 + 3 * H], wih1T[d, k])
        bias1_sb = consts.tile([1, 2 * 3 * H], BF16)
        nc.sync.dma_start(bias1_sb[:], bias1.rearrange("d m -> (d m)"))
        bhhn_sb = consts.tile([2, 2 * H], BF16)  # [dir_row, layer*128+col]
        for l in range(2):
            nc.sync.dma_start(bhhn_sb[:, l * H:(l + 1) * H], bhhn2[l])
        sel2_sb = consts.tile([2, 2 * BL], BF16)
        nc.sync.dma_start(sel2_sb[:], sel2[:])
        wout_sb = consts.tile([H, 2], BF16)
        nc.sync.dma_start(wout_sb[:], woutp[:])
        bout_sb = consts.tile([1, 1], F32)
        nc.sync.dma_start(bout_sb[:], boutp[:])
        ones_sb = consts.tile([1, W], BF16)
        nc.sync.dma_start(ones_sb[:], ones[:])
        hstate = consts.tile([H, 2 * BL], BF16)

        # ---- internal DRAM: layer outputs (backward dir in scan order) ----
        h0f = nc.dram_tensor("h0f", [H, S, BL], BF16, kind="Internal").ap()
        h0b = nc.dram_tensor("h0b", [H, S, BL], BF16, kind="Internal").ap()
        h1f = nc.dram_tensor("h1f", [H, S, BL], BF16, kind="Internal").ap()
        h1b = nc.dram_tensor("h1b", [H, S, BL], BF16, kind="Internal").ap()

        def whh(l, d, g):
            k = (l * 2 + d) * 3 + g
            return whh_sb[:, k * H:(k + 1) * H]

        rec = ExitStack()
        rhsp = rec.enter_context(tc.tile_pool(name="rhsp", bufs=2))
        gxnp = rec.enter_context(tc.tile_pool(name="gxnp", bufs=2))
        ringp = rec.enter_context(tc.tile_pool(name="ringp", bufs=2))
        stepp = rec.enter_context(tc.tile_pool(name="stepp", bufs=3))
        psp = rec.enter_context(tc.tile_pool(name="psp", bufs=1, space="PSUM"))
        psnjp = rec.enter_context(tc.tile_pool(name="psnjp", bufs=2, space="PSUM"))
        psnp = rec.enter_context(tc.tile_pool(name="psnp", bufs=2, space="PSUM"))

        def emit_step(l, j, ring, gxn, rz_ps):
            if j == 0 or STEP_MODE == "nochain":
                hf, hb = hstate[:, 0:BL], hstate[:, BL:2 * BL]
            else:
                hf, hb = ring[:, j - 1, 0:BL], ring[:, j - 1, BL:2 * BL]
            js = slice(j * BL, (j + 1) * BL)

            def rzd(sl):
                return rz_ps[:, sl, js]
            SIG = AF.Copy if "act_copy" in ABLATIONS else AF.Sigmoid
            TANH = AF.Copy if "act_copy" in ABLATIONS else AF.Tanh
            # hn = b_hh_n + W_hh_n @ h  (both dirs) in small psum
            psn = psnp.tile([H, 2 * BL], F32, tag="psn")
            nc.tensor.matmul(psn[:], bhhn_sb[:, l * H:(l + 1) * H], sel2_sb[:],
                             start=True, stop=False, skip_group_check=True)
            if "no_nmm" not in ABLATIONS:
                nc.tensor.matmul(psn[:, 0:BL], whh(l, 0, 2), hf,
                                 start=False, stop=False, skip_group_check=True)
                nc.tensor.matmul(psn[:, BL:2 * BL], whh(l, 1, 2), hb,
                                 start=False, stop=True, skip_group_check=True)
            # r,z gates accumulate onto the prefilled gx slices
            if "no_rzmm" not in ABLATIONS:
                nc.tensor.matmul(rzd(0), whh(l, 0, 0), hf,
                                 start=False, stop=False, skip_group_check=True)
                nc.tensor.matmul(rzd(1), whh(l, 1, 0), hb,
                                 start=False, stop=False, skip_group_check=True)
                nc.tensor.matmul(rzd(2), whh(l, 0, 1), hf,
                                 start=False, stop=False, skip_group_check=True)
                nc.tensor.matmul(rzd(3), whh(l, 1, 1), hb,
                                 start=False, stop=(j == C - 1), skip_group_check=True)
            rz = stepp.tile([H, 4, BL], F32, tag="rz")
            nc.scalar.activation(rz[:], rz_ps[:, :, js], SIG)
            if "no_rn" not in ABLATIONS:
                rn = stepp.tile([H, 2 * BL], F32, tag="rn")
                nc.vector.tensor_mul(rn[:], rz[:, 0:2, :], psn[:])
                arg = stepp.tile([H, 2 * BL], F32, tag="arg")
                nc.vector.tensor_add(arg[:], rn[:], gxn[:, :, js])
                tanh_in = arg
            else:
                tanh_in = None
            n_t = stepp.tile([H, 2 * BL], F32, tag="n")
            if tanh_in is not None:
                nc.scalar.activation(n_t[:], tanh_in[:], TANH)
            else:
                nc.scalar.activation(n_t[:], gxn[:, :, js], TANH)
            if "no_upd" not in ABLATIONS:
                d_t = stepp.tile([H, 2 * BL], F32, tag="d")
                h_prev = (hstate[:, :] if (j == 0 or STEP_MODE == "nochain")
                          else ring[:, j - 1, :])
                nc.vector.tensor_sub(d_t[:], h_prev, n_t[:])
                zd = stepp.tile([H, 2 * BL], F32, tag="zd")
                nc.vector.tensor_mul(zd[:], rz[:, 2:4, :], d_t[:])
                nc.vector.tensor_add(ring[:, j, :], n_t[:], zd[:])
            else:
                nc.vector.tensor_copy(ring[:, j, :], n_t[:])

        def emit_layer(l):
            nc.vector.memset(hstate[:], 0.0)
            h_f_dst, h_b_dst = (h0f, h0b) if l == 0 else (h1f, h1b)
            hints = (mybir.EngineType.PE, mybir.EngineType.DVE) if USE_HINTS else ()
            with tc.For_i(0, NCH, 1, name=f"layer{l}", hint_engines=hints) as i:
                rz_ps = psp.tile([H, 4, W], F32, tag="rzps")
                gxn = gxnp.tile([H, 2, W], F32, tag="gxn")
                ring = ringp.tile([H, C, 2 * BL], BF16, tag="ring")
                # start=True clears the whole 2KB PSUM bank, so it may only be
                # used by the first matmul that touches each bank of rz_ps.
                seen_banks = set()

                def rz_start(sl):
                    bank = sl * W // 512
                    if bank in seen_banks:
                        return False
                    seen_banks.add(bank)
                    return True

                def rz_full(sl):
                    return rz_ps[:, sl, :]

                if l == 0:
                    xf_ch = rhsp.tile([DIN + 1, W], BF16, tag="xf")
                    nc.sync.dma_start(xf_ch[:], xf[:, ds(i * W, W)])
                    xr_ch = rhsp.tile([DIN + 1, W], BF16, tag="xr")
                    nc.sync.dma_start(xr_ch[:], xr[:, ds(i * W, W)])
                    srcs = (xf_ch, xr_ch)
                    for dd, src in enumerate(srcs):
                        for g in range(2):  # r, z -> psum
                            nc.tensor.matmul(
                                rz_full(2 * g + dd),
                                wih0_sb[:, dd * 3 * H + g * H: dd * 3 * H + (g + 1) * H],
                                src[:], start=rz_start(2 * g + dd), stop=False,
                                skip_group_check=True)
                        nj = psnjp.tile([H, W], F32, tag="nj")
                        nc.tensor.matmul(
                            nj[:],
                            wih0_sb[:, dd * 3 * H + 2 * H: dd * 3 * H + 3 * H],
                            src[:], start=True, stop=True, skip_group_check=True)
                        # psum -> sbuf n-ring, split across DVE and ACT
                        hw = W // 2
                        nc.vector.tensor_copy(gxn[:, dd, 0:hw], nj[:, 0:hw])
                        nc.scalar.copy(gxn[:, dd, hw:W], nj[:, hw:W])
                else:
                    # Reversed reads: negative-stride dynamic DRAM APs hang the
                    # device, so read the mirrored chunk forward and reverse on
                    # the SBUF side of the DMA (static negative stride).
                    h0f_v, h0b_v = h0f[:], h0b[:]
                    mir = ds((NCH - 1 - i) * C, C)
                    ff = rhsp.tile([H, C, BL], BF16, tag="ff")
                    nc.sync.dma_start(ff[:], h0f_v[:, ds(i * C, C), :])
                    brv = rhsp.tile([H, C, BL], BF16, tag="brv")
                    nc.sync.dma_start(brv[:, ::-1, :], h0b_v[:, mir, :])
                    frv = rhsp.tile([H, C, BL], BF16, tag="frv")
                    nc.sync.dma_start(frv[:, ::-1, :], h0f_v[:, mir, :])
                    bb = rhsp.tile([H, C, BL], BF16, tag="bb")
                    nc.sync.dma_start(bb[:], h0b_v[:, ds(i * C, C), :])
                    for dd, (rA, rB) in enumerate(((ff, brv), (frv, bb))):
                        base = dd * 2 * 3 * H
                        for g in range(2):
                            dst = rz_full(2 * g + dd)
                            nc.tensor.matmul(dst, wih1_sb[:, base + g * H: base + (g + 1) * H],
                                             rA[:], start=rz_start(2 * g + dd), stop=False,
                                             skip_group_check=True)
                            nc.tensor.matmul(dst, wih1_sb[:, base + 3 * H + g * H: base + 3 * H + (g + 1) * H],
                                             rB[:], start=False, stop=False, skip_group_check=True)
                            nc.tensor.matmul(dst, bias1_sb[:, dd * 3 * H + g * H: dd * 3 * H + (g + 1) * H],
                                             ones_sb[:], start=False, stop=False, skip_group_check=True)
                        nj = psnjp.tile([H, W], F32, tag="nj")
                        nc.tensor.matmul(nj[:], wih1_sb[:, base + 2 * H: base + 3 * H],
                                         rA[:], start=True, stop=False, skip_group_check=True)
                        nc.tensor.matmul(nj[:], wih1_sb[:, base + 3 * H + 2 * H: base + 3 * H + 3 * H],
                                         rB[:], start=False, stop=False, skip_group_check=True)
                        nc.tensor.matmul(nj[:], bias1_sb[:, dd * 3 * H + 2 * H: dd * 3 * H + 3 * H],
                                         ones_sb[:], start=False, stop=True, skip_group_check=True)
                        hw = W // 2
                        nc.vector.tensor_copy(gxn[:, dd, 0:hw], nj[:, 0:hw])
                        nc.scalar.copy(gxn[:, dd, hw:W], nj[:, hw:W])

                if STEP_MODE != "nostep":
                    for j in range(C):
                        emit_step(l, j, ring, gxn, rz_ps)
                else:
                    nc.vector.memset(ring[:], 0.0)

                nc.vector.tensor_copy(hstate[:], ring[:, C - 1, :])
                nc.sync.dma_start(h_f_dst[:][:, ds(i * C, C), :], ring[:, :, 0:BL])
                nc.sync.dma_start(h_b_dst[:][:, ds(i * C, C), :], ring[:, :, BL:2 * BL])

        emit_layer(0)
        if not SKIP_L1:
            emit_layer(1)
        else:
            h1f, h1b = h0f, h0b
        rec.close()

        # ---- head: logits = wout_f . f1[s] + wout_b . b1[s] + bout ----
        if not SKIP_HEAD:
            with tc.tile_pool(name="headp", bufs=3) as hp, \
                 tc.tile_pool(name="headps", bufs=2, space="PSUM") as hps:
                for k in range(NCH):
                    fch = hp.tile([H, W], BF16, tag="fch")
                    nc.sync.dma_start(fch[:], h1f[:][:, k * C:(k + 1) * C, :])
                    bch = hp.tile([H, C, BL], BF16, tag="bch")
                    mk = NCH - 1 - k
                    nc.sync.dma_start(bch[:, ::-1, :], h1b[:][:, mk * C:(mk + 1) * C, :])
                    pso = hps.tile([1, W], F32, tag="pso")
                    nc.tensor.matmul(pso[:], wout_sb[:, 0:1], fch[:],
                                     start=True, stop=False, skip_group_check=True)
                    nc.tensor.matmul(pso[:], wout_sb[:, 1:2], bch[:],
                                     start=False, stop=True, skip_group_check=True)
                    osb = hp.tile([1, W], F32, tag="osb")
                    nc.scalar.activation(osb[:], pso[:], AF.Identity,
                                         bias=bout_sb[0:1, 0:1])
                    nc.sync.dma_start(out_flat[k * W:(k + 1) * W], osb[:])
        stack.close()

    nc.compile()
    return nc


_PROGRAM_CACHE = {}


def _get_program(S=4096, C=64):
    key = (S, C)
    if key not in _PROGRAM_CACHE:
        _PROGRAM_CACHE[key] = build_program(S, C)
    return _PROGRAM_CACHE[key]


def _pack_host_inputs(inputs, S=4096, C=64):
    """Build the per-core input maps from the full problem inputs."""
    W = C * BL
    x = np.asarray(inputs["x"], np.float32)

    def gT(w, g):  # transposed gate block: [in, H]
        return np.ascontiguousarray(np.asarray(w, np.float32)[g * H:(g + 1) * H].T)

    whhT = np.stack([
        gT(inputs[f"whh{l}{d}"], g)
        for l in range(2) for d in "fb" for g in range(3)
    ])  # [12,H,H]

    wih0T = np.zeros((2, DIN + 1, 3 * H), np.float32)
    bhhn2 = np.zeros((2, 2, H), np.float32)
    for di, d in enumerate("fb"):
        wih = np.asarray(inputs[f"wih0{d}"], np.float32)  # [3H, DIN]
        bih = np.asarray(inputs[f"bih0{d}"], np.float32)
        bhh = np.asarray(inputs[f"bhh0{d}"], np.float32)
        wih0T[di, :DIN] = wih.T
        for g in range(3):
            bias = bih[g * H:(g + 1) * H].copy()
            if g < 2:
                bias += bhh[g * H:(g + 1) * H]
            wih0T[di, DIN, g * H:(g + 1) * H] = bias
        bhhn2[0, di] = bhh[2 * H:]

    wih1T = np.zeros((2, 2, H, 3 * H), np.float32)
    bias1 = np.zeros((2, 3 * H), np.float32)
    for di, d in enumerate("fb"):
        wih = np.asarray(inputs[f"wih1{d}"], np.float32)  # [3H, 2H]
        bih = np.asarray(inputs[f"bih1{d}"], np.float32)
        bhh = np.asarray(inputs[f"bhh1{d}"], np.float32)
        for k in range(2):
            for g in range(3):
                wih1T[di, k, :, g * H:(g + 1) * H] = wih[g * H:(g + 1) * H, k * H:(k + 1) * H].T
        for g in range(3):
            bias = bih[g * H:(g + 1) * H].copy()
            if g < 2:
                bias += bhh[g * H:(g + 1) * H]
            bias1[di, g * H:(g + 1) * H] = bias
        bhhn2[1, di] = bhh[2 * H:]

    sel2 = np.zeros((2, 2 * BL), np.float32)
    sel2[0, :BL] = 1.0
    sel2[1, BL:] = 1.0
    woutp = np.zeros((H, 2), np.float32)
    wout = np.asarray(inputs["wout"], np.float32)
    woutp[:, 0] = wout[0, :H]
    woutp[:, 1] = wout[0, H:]
    boutp = np.asarray(inputs["bout"], np.float32).reshape(1, 1)
    ones = np.ones((1, W), np.float32)

    bf = lambda a: np.ascontiguousarray(a.astype(NP_BF16))
    shared = dict(whhT=bf(whhT), wih0T=bf(wih0T), wih1T=bf(wih1T),
                  bias1=bf(bias1), bhhn2=bf(bhhn2), sel2=bf(sel2),
                  woutp=bf(woutp), boutp=boutp, ones=bf(ones))

    in_maps = []
    for c in range(NCORES):
        xc = x[c * BL:(c + 1) * BL]  # [BL, S, DIN]
        arr = np.ones((DIN + 1, S, BL), np.float32)
        arr[:DIN] = xc.transpose(2, 1, 0)
        xfm = bf(arr.reshape(DIN + 1, S * BL))
        xrm = bf(np.ascontiguousarray(arr[:, ::-1, :].reshape(DIN + 1, S * BL)))
        in_maps.append(dict(shared, xf=xfm, xr=xrm))
    return in_maps


def kernel(**inputs) -> np.ndarray:
    from concourse import bass_utils
    S, C = 4096, 64
    nc = _get_program(S, C)
    in_maps = _pack_host_inputs(inputs, S, C)
    res = bass_utils.run_bass_kernel_spmd(nc, in_maps, core_ids=list(range(NCORES)))
    outs = [r["out"] for r in res.results]  # each [S, BL]
    return np.concatenate([o.T for o in outs], axis=0).astype(np.float32)


# revision 9
# speedup vs baseline: 2.1677x; 1.5859x over previous
"""Trainium2 Bass kernel for a 2-layer bidirectional GRU + linear head.

Problem: B=64, S=4096, D_IN=7, H=128, PyTorch gate order (r, z, n).
Sharding: data-parallel over batch across 8 NeuronCores (8 rows each).

Per-core design (all layouts keep H=128 on the SBUF partition axis):
  - The sequence is processed in chunks of C=64 steps. For each chunk the
    input-gate projections gx = W_ih @ x (+ biases) for the r,z gates of both
    directions are computed by bulk matmuls directly into a 4-bank PSUM tile
    [128, 4, C*8]; the per-step recurrent matmuls (W_hh @ h) then accumulate
    onto their 8-column slice (start=False), so sigmoid reads (xr+hr, xz+hz)
    straight out of PSUM with zero staging ops.
  - The n-gate projections go to an SBUF ring (xn must not receive W_hh@h
    before the r* multiply); b_hh_n is staged into a small PSUM tile with a
    rank-2 matmul, and W_hh_n@h accumulates there.
  - Both directions are packed into the free dim of every elementwise op
    (columns 0:8 forward, 8:16 backward); the backward direction consumes a
    host-reversed copy of x so all its tensors are in scan order ("u" order),
    and the reversal is applied via negative-stride APs when layer 1 / the
    head need time-aligned pairs.
  - The hidden-state ring [128, C, 16] doubles as the output buffer: the
    final h' add of each step writes the ring slot, which the next step's
    matmuls read as rhs and which is DMA'd to DRAM per chunk.
  - All matmul operands are bf16 (PSUM accumulation and every elementwise op
    stay fp32): fp32 matmuls are decomposed by HW into TWO ldweights+matmul
    passes (fp32_mode LOW/HIGH) and disable Fast Weight Load, roughly
    doubling PE occupancy, and fp32 PE activity triggers the HAM power
    throttle (50% duty cycle). bf16 halves the PE instruction stream.
"""

import numpy as np
import ml_dtypes

import concourse.bass as bass
import concourse.tile as tile
from concourse import bacc, mybir
from concourse.bass import ds

F32 = mybir.dt.float32
BF16 = mybir.dt.bfloat16
NP_BF16 = ml_dtypes.bfloat16
AF = mybir.ActivationFunctionType

H = 128
DIN = 7
B = 64
NCORES = 8
BL = B // NCORES  # batch rows per core


STEP_MODE = "full"   # "full" | "nochain" (steps read hstate, no serial dep) | "nostep"
ABLATIONS = set()
USE_HINTS = True
SKIP_L1 = False      # emit only layer 0; head reads h0f/h0b
SKIP_HEAD = False    # skip the head phase (out left zero)


def build_program(S=4096, C=64, n_cores=NCORES):
    """Build the per-core Bass program."""
    NCH = S // C
    W = C * BL  # chunk columns (= matmul moving-dim), 512 for C=64
    nc = bacc.Bacc("TRN2", target_bir_lowering=False, debug=False)

    # ---- DRAM I/O ----
    xf = nc.dram_tensor("xf", [DIN + 1, S * BL], BF16, kind="ExternalInput").ap()
    xr = nc.dram_tensor("xr", [DIN + 1, S * BL], BF16, kind="ExternalInput").ap()
    whhT = nc.dram_tensor("whhT", [12, H, H], BF16, kind="ExternalInput").ap()
    wih0T = nc.dram_tensor("wih0T", [2, DIN + 1, 3 * H], BF16, kind="ExternalInput").ap()
    wih1T = nc.dram_tensor("wih1T", [2, 2, H, 3 * H], BF16, kind="ExternalInput").ap()
    bias1 = nc.dram_tensor("bias1", [2, 3 * H], BF16, kind="ExternalInput").ap()
    bhhn2 = nc.dram_tensor("bhhn2", [2, 2, H], BF16, kind="ExternalInput").ap()
    sel2 = nc.dram_tensor("sel2", [2, 2 * BL], BF16, kind="ExternalInput").ap()
    woutp = nc.dram_tensor("woutp", [H, 2], BF16, kind="ExternalInput").ap()
    boutp = nc.dram_tensor("boutp", [1, 1], F32, kind="ExternalInput").ap()
    ones = nc.dram_tensor("ones", [1, W], BF16, kind="ExternalInput").ap()
    out = nc.dram_tensor("out", [S, BL], F32, kind="ExternalOutput").ap()
    out_flat = out.rearrange("s b -> (s b)")

    with tile.TileContext(nc) as tc:
        from contextlib import ExitStack

        stack = ExitStack()
        consts = stack.enter_context(tc.tile_pool(name="consts", bufs=1))

        # ---- persistent SBUF constants (all bf16 matmul operands) ----
        whh_sb = consts.tile([H, 12 * H], BF16)  # (l,d,g) blocks of 128 cols
        for k in range(12):
            nc.sync.dma_start(whh_sb[:, k * H:(k + 1) * H], whhT[k])
        wih0_sb = consts.tile([DIN + 1, 2 * 3 * H], BF16)
        for d in range(2):
            nc.sync.dma_start(wih0_sb[:, d * 3 * H:(d + 1) * 3 * H], wih0T[d])
        wih1_sb = consts.tile([H, 4 * 3 * H], BF16)  # (d,k) blocks of 384 cols
        for d in range(2):
            for k in range(2):
                c0 = (d * 2 + k) * 3 * H
                nc.sync.dma_start(wih1_sb[:, c0:c0 + 3 * H], wih1T[d, k])
        bias1_sb = consts.tile([1, 2 * 3 * H], BF16)
        nc.sync.dma_start(bias1_sb[:], bias1.rearrange("d m -> (d m)"))
        bhhn_sb = consts.tile([2, 2 * H], BF16)  # [dir_row, layer*128+col]
        for l in range(2):
            nc.sync.dma_start(bhhn_sb[:, l * H:(l + 1) * H], bhhn2[l])
        sel2_sb = consts.tile([2, 2 * BL], BF16)
        nc.sync.dma_start(sel2_sb[:], sel2[:])
        wout_sb = consts.tile([H, 2], BF16)
        nc.sync.dma_start(wout_sb[:], woutp[:])
        bout_sb = consts.tile([1, 1], F32)
        nc.sync.dma_start(bout_sb[:], boutp[:])
        ones_sb = consts.tile([1, W], BF16)
        nc.sync.dma_start(ones_sb[:], ones[:])
        hstate = consts.tile([H, 2 * BL], BF16)

        # ---- internal DRAM: layer outputs (backward dir in scan order) ----
        h0f = nc.dram_tensor("h0f", [H, S, BL], BF16, kind="Internal").ap()
        h0b = nc.dram_tensor("h0b", [H, S, BL], BF16, kind="Internal").ap()
        h1f = nc.dram_tensor("h1f", [H, S, BL], BF16, kind="Internal").ap()
        h1b = nc.dram_tensor("h1b", [H, S, BL], BF16, kind="Internal").ap()

        def whh(l, d, g):
            k = (l * 2 + d) * 3 + g
            return whh_sb[:, k * H:(k + 1) * H]

        rec = ExitStack()
        rhsp = rec.enter_context(tc.tile_pool(name="rhsp", bufs=2))
        gxnp = rec.enter_context(tc.tile_pool(name="gxnp", bufs=2))
        ringp = rec.enter_context(tc.tile_pool(name="ringp", bufs=2))
        stepp = rec.enter_context(tc.tile_pool(name="stepp", bufs=3))
        psp = rec.enter_context(tc.tile_pool(name="psp", bufs=1, space="PSUM"))
        psnjp = rec.enter_context(tc.tile_pool(name="psnjp", bufs=2, space="PSUM"))
        psnp = rec.enter_context(tc.tile_pool(name="psnp", bufs=2, space="PSUM"))

        def emit_step(l, j, ring, gxn, rz_ps):
            if j == 0 or STEP_MODE == "nochain":
                hf, hb = hstate[:, 0:BL], hstate[:, BL:2 * BL]
            else:
                hf, hb = ring[:, 0, j - 1, :], ring[:, 1, j - 1, :]
            js = slice(j * BL, (j + 1) * BL)

            def rzd(sl):
                return rz_ps[:, sl, js]
            SIG = AF.Copy if "act_copy" in ABLATIONS else AF.Sigmoid
            TANH = AF.Copy if "act_copy" in ABLATIONS else AF.Tanh
            # hn = b_hh_n + W_hh_n @ h  (both dirs) in small psum
            psn = psnp.tile([H, 2 * BL], F32, tag="psn")
            nc.tensor.matmul(psn[:], bhhn_sb[:, l * H:(l + 1) * H], sel2_sb[:],
                             start=True, stop=False, skip_group_check=True)
            if "no_nmm" not in ABLATIONS:
                nc.tensor.matmul(psn[:, 0:BL], whh(l, 0, 2), hf,
                                 start=False, stop=False, skip_group_check=True)
                nc.tensor.matmul(psn[:, BL:2 * BL], whh(l, 1, 2), hb,
                                 start=False, stop=True, skip_group_check=True)
            # r,z gates accumulate onto the prefilled gx slices
            if "no_rzmm" not in ABLATIONS:
                nc.tensor.matmul(rzd(0), whh(l, 0, 0), hf,
                                 start=False, stop=False, skip_group_check=True)
                nc.tensor.matmul(rzd(1), whh(l, 1, 0), hb,
                                 start=False, stop=False, skip_group_check=True)
                nc.tensor.matmul(rzd(2), whh(l, 0, 1), hf,
                                 start=False, stop=False, skip_group_check=True)
                nc.tensor.matmul(rzd(3), whh(l, 1, 1), hb,
                                 start=False, stop=(j == C - 1), skip_group_check=True)
            rz = stepp.tile([H, 4, BL], F32, tag="rz")
            nc.scalar.activation(rz[:], rz_ps[:, :, js], SIG)
            if "no_rn" not in ABLATIONS:
                rn = stepp.tile([H, 2 * BL], F32, tag="rn")
                nc.vector.tensor_mul(rn[:], rz[:, 0:2, :], psn[:])
                arg = stepp.tile([H, 2 * BL], F32, tag="arg")
                nc.vector.tensor_add(arg[:], rn[:], gxn[:, :, js])
                tanh_in = arg
            else:
                tanh_in = None
            n_t = stepp.tile([H, 2 * BL], F32, tag="n")
            if tanh_in is not None:
                nc.scalar.activation(n_t[:], tanh_in[:], TANH)
            else:
                nc.scalar.activation(n_t[:], gxn[:, :, js], TANH)
            if "no_upd" not in ABLATIONS:
                d_t = stepp.tile([H, 2 * BL], F32, tag="d")
                h_prev = (hstate[:, :] if (j == 0 or STEP_MODE == "nochain")
                          else ring[:, :, j - 1, :])
                nc.vector.tensor_sub(d_t[:], h_prev, n_t[:])
                zd = stepp.tile([H, 2 * BL], F32, tag="zd")
                nc.vector.tensor_mul(zd[:], rz[:, 2:4, :], d_t[:])
                nc.vector.tensor_add(ring[:, :, j, :], n_t[:], zd[:])
            else:
                nc.vector.tensor_copy(ring[:, :, j, :], n_t[:])

        def emit_layer(l):
            nc.vector.memset(hstate[:], 0.0)
            h_f_dst, h_b_dst = (h0f, h0b) if l == 0 else (h1f, h1b)
            hints = (mybir.EngineType.PE, mybir.EngineType.DVE) if USE_HINTS else ()
            with tc.For_i(0, NCH, 1, name=f"layer{l}", hint_engines=hints,
                          staggered_reset=True) as i:
                rz_ps = psp.tile([H, 4, W], F32, tag="rzps")
                gxn = gxnp.tile([H, 2, W], F32, tag="gxn")
                ring = ringp.tile([H, 2, C, BL], BF16, tag="ring")
                # start=True clears the whole 2KB PSUM bank, so it may only be
                # used by the first matmul that touches each bank of rz_ps.
                seen_banks = set()

                def rz_start(sl):
                    bank = sl * W // 512
                    if bank in seen_banks:
                        return False
                    seen_banks.add(bank)
                    return True

                def rz_full(sl):
                    return rz_ps[:, sl, :]

                if l == 0:
                    xf_ch = rhsp.tile([DIN + 1, W], BF16, tag="xf")
                    nc.sync.dma_start(xf_ch[:], xf[:, ds(i * W, W)])
                    xr_ch = rhsp.tile([DIN + 1, W], BF16, tag="xr")
                    nc.sync.dma_start(xr_ch[:], xr[:, ds(i * W, W)])
                    srcs = (xf_ch, xr_ch)
                    for dd, src in enumerate(srcs):
                        for g in range(2):  # r, z -> psum
                            nc.tensor.matmul(
                                rz_full(2 * g + dd),
                                wih0_sb[:, dd * 3 * H + g * H: dd * 3 * H + (g + 1) * H],
                                src[:], start=rz_start(2 * g + dd), stop=False,
                                skip_group_check=True)
                        nj = psnjp.tile([H, W], F32, tag="nj")
                        nc.tensor.matmul(
                            nj[:],
                            wih0_sb[:, dd * 3 * H + 2 * H: dd * 3 * H + 3 * H],
                            src[:], start=True, stop=True, skip_group_check=True)
                        # psum -> sbuf n-ring, split across DVE and ACT
                        hw = W // 2
                        nc.vector.tensor_copy(gxn[:, dd, 0:hw], nj[:, 0:hw])
                        nc.scalar.copy(gxn[:, dd, hw:W], nj[:, hw:W])
                else:
                    # All four chunk loads are contiguous (1KB/partition);
                    # time-reversal is applied via negative-stride SBUF views
                    # on the matmul rhs instead of per-row DMA descriptors.
                    h0f_v, h0b_v = h0f[:], h0b[:]
                    mir = ds((NCH - 1 - i) * C, C)
                    ff = rhsp.tile([H, C, BL], BF16, tag="ff")
                    nc.sync.dma_start(ff[:], h0f_v[:, ds(i * C, C), :])
                    bm = rhsp.tile([H, C, BL], BF16, tag="bm")
                    nc.sync.dma_start(bm[:], h0b_v[:, mir, :])
                    fm = rhsp.tile([H, C, BL], BF16, tag="fm")
                    nc.sync.dma_start(fm[:], h0f_v[:, mir, :])
                    bb = rhsp.tile([H, C, BL], BF16, tag="bb")
                    nc.sync.dma_start(bb[:], h0b_v[:, ds(i * C, C), :])
                    brv = bm[:, ::-1, :]
                    frv = fm[:, ::-1, :]
                    for dd, (rA, rB) in enumerate(((ff, brv), (frv, bb))):
                        base = dd * 2 * 3 * H
                        for g in range(2):
                            dst = rz_full(2 * g + dd)
                            nc.tensor.matmul(dst, wih1_sb[:, base + g * H: base + (g + 1) * H],
                                             rA[:], start=rz_start(2 * g + dd), stop=False,
                                             skip_group_check=True)
                            nc.tensor.matmul(dst, wih1_sb[:, base + 3 * H + g * H: base + 3 * H + (g + 1) * H],
                                             rB[:], start=False, stop=False, skip_group_check=True)
                            nc.tensor.matmul(dst, bias1_sb[:, dd * 3 * H + g * H: dd * 3 * H + (g + 1) * H],
                                             ones_sb[:], start=False, stop=False, skip_group_check=True)
                        nj = psnjp.tile([H, W], F32, tag="nj")
                        nc.tensor.matmul(nj[:], wih1_sb[:, base + 2 * H: base + 3 * H],
                                         rA[:], start=True, stop=False, skip_group_check=True)
                        nc.tensor.matmul(nj[:], wih1_sb[:, base + 3 * H + 2 * H: base + 3 * H + 3 * H],
                                         rB[:], start=False, stop=False, skip_group_check=True)
                        nc.tensor.matmul(nj[:], bias1_sb[:, dd * 3 * H + 2 * H: dd * 3 * H + 3 * H],
                                         ones_sb[:], start=False, stop=True, skip_group_check=True)
                        hw = W // 2
                        nc.vector.tensor_copy(gxn[:, dd, 0:hw], nj[:, 0:hw])
                        nc.scalar.copy(gxn[:, dd, hw:W], nj[:, hw:W])

                if STEP_MODE != "nostep":
                    for j in range(C):
                        emit_step(l, j, ring, gxn, rz_ps)
                else:
                    nc.vector.memset(ring[:], 0.0)

                nc.vector.tensor_copy(hstate[:], ring[:, :, C - 1, :])
                nc.sync.dma_start(h_f_dst[:][:, ds(i * C, C), :], ring[:, 0])
                nc.sync.dma_start(h_b_dst[:][:, ds(i * C, C), :], ring[:, 1])

        emit_layer(0)
        if not SKIP_L1:
            emit_layer(1)
        else:
            h1f, h1b = h0f, h0b
        rec.close()

        # ---- head: logits = wout_f . f1[s] + wout_b . b1[s] + bout ----
        if not SKIP_HEAD:
            with tc.tile_pool(name="headp", bufs=3) as hp, \
                 tc.tile_pool(name="headps", bufs=2, space="PSUM") as hps:
                for k in range(NCH):
                    fch = hp.tile([H, W], BF16, tag="fch")
                    nc.sync.dma_start(fch[:], h1f[:][:, k * C:(k + 1) * C, :])
                    bch = hp.tile([H, C, BL], BF16, tag="bch")
                    mk = NCH - 1 - k
                    nc.sync.dma_start(bch[:], h1b[:][:, mk * C:(mk + 1) * C, :])
                    pso = hps.tile([1, W], F32, tag="pso")
                    nc.tensor.matmul(pso[:], wout_sb[:, 0:1], fch[:],
                                     start=True, stop=False, skip_group_check=True)
                    nc.tensor.matmul(pso[:], wout_sb[:, 1:2], bch[:, ::-1, :],
                                     start=False, stop=True, skip_group_check=True)
                    osb = hp.tile([1, W], F32, tag="osb")
                    nc.scalar.activation(osb[:], pso[:], AF.Identity,
                                         bias=bout_sb[0:1, 0:1])
                    nc.sync.dma_start(out_flat[k * W:(k + 1) * W], osb[:])
        stack.close()

    nc.compile()
    return nc


_PROGRAM_CACHE = {}


def _get_program(S=4096, C=64):
    key = (S, C)
    if key not in _PROGRAM_CACHE:
        _PROGRAM_CACHE[key] = build_program(S, C)
    return _PROGRAM_CACHE[key]


def _pack_host_inputs(inputs, S=4096, C=64):
    """Build the per-core input maps from the full problem inputs."""
    W = C * BL
    x = np.asarray(inputs["x"], np.float32)

    def gT(w, g):  # transposed gate block: [in, H]
        return np.ascontiguousarray(np.asarray(w, np.float32)[g * H:(g + 1) * H].T)

    whhT = np.stack([
        gT(inputs[f"whh{l}{d}"], g)
        for l in range(2) for d in "fb" for g in range(3)
    ])  # [12,H,H]

    wih0T = np.zeros((2, DIN + 1, 3 * H), np.float32)
    bhhn2 = np.zeros((2, 2, H), np.float32)
    for di, d in enumerate("fb"):
        wih = np.asarray(inputs[f"wih0{d}"], np.float32)  # [3H, DIN]
        bih = np.asarray(inputs[f"bih0{d}"], np.float32)
        bhh = np.asarray(inputs[f"bhh0{d}"], np.float32)
        wih0T[di, :DIN] = wih.T
        for g in range(3):
            bias = bih[g * H:(g + 1) * H].copy()
            if g < 2:
                bias += bhh[g * H:(g + 1) * H]
            wih0T[di, DIN, g * H:(g + 1) * H] = bias
        bhhn2[0, di] = bhh[2 * H:]

    wih1T = np.zeros((2, 2, H, 3 * H), np.float32)
    bias1 = np.zeros((2, 3 * H), np.float32)
    for di, d in enumerate("fb"):
        wih = np.asarray(inputs[f"wih1{d}"], np.float32)  # [3H, 2H]
        bih = np.asarray(inputs[f"bih1{d}"], np.float32)
        bhh = np.asarray(inputs[f"bhh1{d}"], np.float32)
        for k in range(2):
            for g in range(3):
                wih1T[di, k, :, g * H:(g + 1) * H] = wih[g * H:(g + 1) * H, k * H:(k + 1) * H].T
        for g in range(3):
            bias = bih[g * H:(g + 1) * H].copy()
            if g < 2:
                bias += bhh[g * H:(g + 1) * H]
            bias1[di, g * H:(g + 1) * H] = bias
        bhhn2[1, di] = bhh[2 * H:]

    sel2 = np.zeros((2, 2 * BL), np.float32)
    sel2[0, :BL] = 1.0
    sel2[1, BL:] = 1.0
    woutp = np.zeros((H, 2), np.float32)
    wout = np.asarray(inputs["wout"], np.float32)
    woutp[:, 0] = wout[0, :H]
    woutp[:, 1] = wout[0, H:]
    boutp = np.asarray(inputs["bout"], np.float32).reshape(1, 1)
    ones = np.ones((1, W), np.float32)

    bf = lambda a: np.ascontiguousarray(a.astype(NP_BF16))
    shared = dict(whhT=bf(whhT), wih0T=bf(wih0T), wih1T=bf(wih1T),
                  bias1=bf(bias1), bhhn2=bf(bhhn2), sel2=bf(sel2),
                  woutp=bf(woutp), boutp=boutp, ones=bf(ones))

    in_maps = []
    for c in range(NCORES):
        xc = x[c * BL:(c + 1) * BL]  # [BL, S, DIN]
        arr = np.ones((DIN + 1, S, BL), np.float32)
        arr[:DIN] = xc.transpose(2, 1, 0)
        xfm = bf(arr.reshape(DIN + 1, S * BL))
        xrm = bf(np.ascontiguousarray(arr[:, ::-1, :].reshape(DIN + 1, S * BL)))
        in_maps.append(dict(shared, xf=xfm, xr=xrm))
    return in_maps


def kernel(**inputs) -> np.ndarray:
    from concourse import bass_utils
    S, C = 4096, 64
    nc = _get_program(S, C)
    in_maps = _pack_host_inputs(inputs, S, C)
    res = bass_utils.run_bass_kernel_spmd(nc, in_maps, core_ids=list(range(NCORES)))
    outs = [r["out"] for r in res.results]  # each [S, BL]
    return np.concatenate([o.T for o in outs], axis=0).astype(np.float32)


# revision 14
# speedup vs baseline: 2.3954x; 1.1051x over previous
"""Trainium2 Bass kernel for a 2-layer bidirectional GRU + linear head.

Problem: B=64, S=4096, D_IN=7, H=128, PyTorch gate order (r, z, n).
Sharding: data-parallel over batch across 8 NeuronCores (8 rows each).

Per-core design (all layouts keep H=128 on the SBUF partition axis):
  - The sequence is processed in chunks of C=64 steps. For each chunk the
    input-gate projections gx = W_ih @ x (+ biases) for the r,z gates of both
    directions are computed by bulk matmuls directly into a 4-bank PSUM tile
    [128, 4, C*8]; the per-step recurrent matmuls (W_hh @ h) then accumulate
    onto their 8-column slice (start=False), so sigmoid reads (xr+hr, xz+hz)
    straight out of PSUM with zero staging ops.
  - The n-gate projections go to an SBUF ring (xn must not receive W_hh@h
    before the r* multiply); b_hh_n is staged into a small PSUM tile with a
    rank-2 matmul, and W_hh_n@h accumulates there.
  - Both directions are packed into the free dim of every elementwise op
    (columns 0:8 forward, 8:16 backward); the backward direction consumes a
    host-reversed copy of x so all its tensors are in scan order ("u" order),
    and the reversal is applied via negative-stride APs when layer 1 / the
    head need time-aligned pairs.
  - The hidden-state ring [128, C, 16] doubles as the output buffer: the
    final h' add of each step writes the ring slot, which the next step's
    matmuls read as rhs and which is DMA'd to DRAM per chunk.
  - All matmul operands are bf16 (PSUM accumulation and every elementwise op
    stay fp32): fp32 matmuls are decomposed by HW into TWO ldweights+matmul
    passes (fp32_mode LOW/HIGH) and disable Fast Weight Load, roughly
    doubling PE occupancy, and fp32 PE activity triggers the HAM power
    throttle (50% duty cycle). bf16 halves the PE instruction stream.
"""

import numpy as np
import ml_dtypes

import concourse.bass as bass
import concourse.tile as tile
from concourse import bacc, mybir
from concourse.bass import ds

F32 = mybir.dt.float32
BF16 = mybir.dt.bfloat16
NP_BF16 = ml_dtypes.bfloat16
AF = mybir.ActivationFunctionType

H = 128
DIN = 7
B = 64
NCORES = 8
BL = B // NCORES  # batch rows per core


STEP_MODE = "full"   # "full" | "nochain" (steps read hstate, no serial dep) | "nostep"
ABLATIONS = set()
USE_HINTS = True
SKIP_L1 = False      # emit only layer 0; head reads h0f/h0b
SKIP_HEAD = False    # skip the head phase (out left zero)


def build_program(S=4096, C=64, n_cores=NCORES):
    """Build the per-core Bass program."""
    NCH = S // C
    W = C * BL  # chunk columns (= matmul moving-dim), 512 for C=64
    nc = bacc.Bacc("TRN2", target_bir_lowering=False, debug=False)

    # ---- DRAM I/O ----
    xf = nc.dram_tensor("xf", [DIN + 1, S * BL], BF16, kind="ExternalInput").ap()
    xr = nc.dram_tensor("xr", [DIN + 1, S * BL], BF16, kind="ExternalInput").ap()
    whhT = nc.dram_tensor("whhT", [12, H, H], BF16, kind="ExternalInput").ap()
    wih0T = nc.dram_tensor("wih0T", [2, DIN + 1, 3 * H], BF16, kind="ExternalInput").ap()
    wih1T = nc.dram_tensor("wih1T", [2, 2, H, 3 * H], BF16, kind="ExternalInput").ap()
    bias1 = nc.dram_tensor("bias1", [2, 3 * H], BF16, kind="ExternalInput").ap()
    bhhn2 = nc.dram_tensor("bhhn2", [2, 2, H], BF16, kind="ExternalInput").ap()
    sel2 = nc.dram_tensor("sel2", [2, 2 * BL], BF16, kind="ExternalInput").ap()
    woutp = nc.dram_tensor("woutp", [H, 2], BF16, kind="ExternalInput").ap()
    boutp = nc.dram_tensor("boutp", [1, 1], F32, kind="ExternalInput").ap()
    ones = nc.dram_tensor("ones", [1, W], BF16, kind="ExternalInput").ap()
    out = nc.dram_tensor("out", [S, BL], F32, kind="ExternalOutput").ap()
    out_flat = out.rearrange("s b -> (s b)")

    with tile.TileContext(nc) as tc:
        from contextlib import ExitStack

        stack = ExitStack()
        consts = stack.enter_context(tc.tile_pool(name="consts", bufs=1))

        # ---- persistent SBUF constants (all bf16 matmul operands) ----
        whh_sb = consts.tile([H, 12 * H], BF16)  # (l,d,g) blocks of 128 cols
        for k in range(12):
            nc.sync.dma_start(whh_sb[:, k * H:(k + 1) * H], whhT[k])
        wih0_sb = consts.tile([DIN + 1, 2 * 3 * H], BF16)
        for d in range(2):
            nc.sync.dma_start(wih0_sb[:, d * 3 * H:(d + 1) * 3 * H], wih0T[d])
        wih1_sb = consts.tile([H, 4 * 3 * H], BF16)  # (d,k) blocks of 384 cols
        for d in range(2):
            for k in range(2):
                c0 = (d * 2 + k) * 3 * H
                nc.sync.dma_start(wih1_sb[:, c0:c0 + 3 * H], wih1T[d, k])
        bias1_sb = consts.tile([1, 2 * 3 * H], BF16)
        nc.sync.dma_start(bias1_sb[:], bias1.rearrange("d m -> (d m)"))
        bhhn_sb = consts.tile([2, 2 * H], BF16)  # [dir_row, layer*128+col]
        for l in range(2):
            nc.sync.dma_start(bhhn_sb[:, l * H:(l + 1) * H], bhhn2[l])
        sel2_sb = consts.tile([2, 2 * BL], BF16)
        nc.sync.dma_start(sel2_sb[:], sel2[:])
        wout_sb = consts.tile([H, 2], BF16)
        nc.sync.dma_start(wout_sb[:], woutp[:])
        bout_sb = consts.tile([1, 1], F32)
        nc.sync.dma_start(bout_sb[:], boutp[:])
        ones_sb = consts.tile([1, W], BF16)
        nc.sync.dma_start(ones_sb[:], ones[:])
        hstate = consts.tile([H, 2 * BL], BF16)

        # ---- internal DRAM: layer outputs (backward dir in scan order) ----
        h0f = nc.dram_tensor("h0f", [H, S, BL], BF16, kind="Internal").ap()
        h0b = nc.dram_tensor("h0b", [H, S, BL], BF16, kind="Internal").ap()
        h1f = nc.dram_tensor("h1f", [H, S, BL], BF16, kind="Internal").ap()
        h1b = nc.dram_tensor("h1b", [H, S, BL], BF16, kind="Internal").ap()

        def whh(l, d, g):
            k = (l * 2 + d) * 3 + g
            return whh_sb[:, k * H:(k + 1) * H]

        rec = ExitStack()
        rhsp = rec.enter_context(tc.tile_pool(name="rhsp", bufs=2))
        gxnp = rec.enter_context(tc.tile_pool(name="gxnp", bufs=2))
        ringp = rec.enter_context(tc.tile_pool(name="ringp", bufs=2))
        stepp = rec.enter_context(tc.tile_pool(name="stepp", bufs=3))
        psp = rec.enter_context(tc.tile_pool(name="psp", bufs=1, space="PSUM"))
        psnjp = rec.enter_context(tc.tile_pool(name="psnjp", bufs=2, space="PSUM"))
        psnp = rec.enter_context(tc.tile_pool(name="psnp", bufs=2, space="PSUM"))

        def emit_step(l, j, ring, gxn, rz_ps):
            if j == 0 or STEP_MODE == "nochain":
                hf, hb = hstate[:, 0:BL], hstate[:, BL:2 * BL]
            else:
                hf, hb = ring[:, 0, j - 1, :], ring[:, 1, j - 1, :]
            js = slice(j * BL, (j + 1) * BL)

            def rzd(sl):
                return rz_ps[:, sl, js]
            SIG = AF.Copy if "act_copy" in ABLATIONS else AF.Sigmoid
            TANH = AF.Copy if "act_copy" in ABLATIONS else AF.Tanh
            # hn = b_hh_n + W_hh_n @ h  (both dirs) in small psum
            psn = psnp.tile([H, 2 * BL], F32, tag="psn")
            nc.tensor.matmul(psn[:], bhhn_sb[:, l * H:(l + 1) * H], sel2_sb[:],
                             start=True, stop=False, skip_group_check=True)
            if "no_nmm" not in ABLATIONS:
                nc.tensor.matmul(psn[:, 0:BL], whh(l, 0, 2), hf,
                                 start=False, stop=False, skip_group_check=True)
                nc.tensor.matmul(psn[:, BL:2 * BL], whh(l, 1, 2), hb,
                                 start=False, stop=True, skip_group_check=True)
            # r,z gates accumulate onto the prefilled gx slices
            if "no_rzmm" not in ABLATIONS:
                nc.tensor.matmul(rzd(0), whh(l, 0, 0), hf,
                                 start=False, stop=False, skip_group_check=True)
                nc.tensor.matmul(rzd(1), whh(l, 1, 0), hb,
                                 start=False, stop=False, skip_group_check=True)
                nc.tensor.matmul(rzd(2), whh(l, 0, 1), hf,
                                 start=False, stop=False, skip_group_check=True)
                nc.tensor.matmul(rzd(3), whh(l, 1, 1), hb,
                                 start=False, stop=(j == C - 1), skip_group_check=True)
            # rz slices: 0:2 = r (f,b); 2:4 = zbar = 1-z (z weights negated on
            # host, so sigmoid yields zbar directly).
            rz = stepp.tile([H, 4, BL], F32, tag="rz")
            nc.scalar.activation(rz[:], rz_ps[:, :, js], SIG)
            h_prev = (hstate[:, :] if (j == 0 or STEP_MODE == "nochain")
                      else ring[:, :, j - 1, :])
            if "no_rn" not in ABLATIONS:
                rn = stepp.tile([H, 2 * BL], F32, tag="rn")
                nc.vector.tensor_mul(rn[:], rz[:, 0:2, :], psn[:])
                arg = stepp.tile([H, 2 * BL], F32, tag="arg")
                nc.vector.tensor_add(arg[:], rn[:], gxn[:, :, js])
                tanh_in = arg
            else:
                tanh_in = None
            # t2 = h - zbar*h runs on DVE during the tanh: h' = zbar*n + t2
            t1 = stepp.tile([H, 2 * BL], F32, tag="t1")
            nc.vector.tensor_mul(t1[:], rz[:, 2:4, :], h_prev)
            t2 = stepp.tile([H, 2 * BL], F32, tag="t2")
            nc.vector.tensor_sub(t2[:], h_prev, t1[:])
            n_t = stepp.tile([H, 2 * BL], F32, tag="n")
            if tanh_in is not None:
                nc.scalar.activation(n_t[:], tanh_in[:], TANH)
            else:
                nc.scalar.activation(n_t[:], gxn[:, :, js], TANH)
            if "no_upd" not in ABLATIONS:
                nz = stepp.tile([H, 2 * BL], F32, tag="nz")
                nc.vector.tensor_mul(nz[:], rz[:, 2:4, :], n_t[:])
                nc.vector.tensor_add(ring[:, :, j, :], nz[:], t2[:])
            else:
                nc.vector.tensor_copy(ring[:, :, j, :], n_t[:])

        def emit_layer(l):
            nc.vector.memset(hstate[:], 0.0)
            h_f_dst, h_b_dst = (h0f, h0b) if l == 0 else (h1f, h1b)
            hints = (mybir.EngineType.PE, mybir.EngineType.DVE) if USE_HINTS else ()
            with tc.For_i(0, NCH, 1, name=f"layer{l}", hint_engines=hints,
                          staggered_reset=True) as i:
                rz_ps = psp.tile([H, 4, W], F32, tag="rzps")
                gxn = gxnp.tile([H, 2, W], F32, tag="gxn")
                ring = ringp.tile([H, 2, C, BL], BF16, tag="ring")
                # start=True clears the whole 2KB PSUM bank, so it may only be
                # used by the first matmul that touches each bank of rz_ps.
                seen_banks = set()

                def rz_start(sl):
                    bank = sl * W // 512
                    if bank in seen_banks:
                        return False
                    seen_banks.add(bank)
                    return True

                def rz_full(sl):
                    return rz_ps[:, sl, :]

                if l == 0:
                    xf_ch = rhsp.tile([DIN + 1, W], BF16, tag="xf")
                    nc.sync.dma_start(xf_ch[:], xf[:, ds(i * W, W)])
                    xr_ch = rhsp.tile([DIN + 1, W], BF16, tag="xr")
                    nc.sync.dma_start(xr_ch[:], xr[:, ds(i * W, W)])
                    srcs = (xf_ch, xr_ch)
                    for dd, src in enumerate(srcs):
                        for g in range(2):  # r, z -> psum
                            nc.tensor.matmul(
                                rz_full(2 * g + dd),
                                wih0_sb[:, dd * 3 * H + g * H: dd * 3 * H + (g + 1) * H],
                                src[:], start=rz_start(2 * g + dd), stop=False,
                                skip_group_check=True)
                        nj = psnjp.tile([H, W], F32, tag="nj")
                        nc.tensor.matmul(
                            nj[:],
                            wih0_sb[:, dd * 3 * H + 2 * H: dd * 3 * H + 3 * H],
                            src[:], start=True, stop=True, skip_group_check=True)
                        # psum -> sbuf n-ring, split across DVE and ACT
                        # both halves on DVE: scalar-engine copies evict the
                        # sigmoid/tanh activation tables (1.3us reload each)
                        hw = W // 2
                        nc.vector.tensor_copy(gxn[:, dd, 0:hw], nj[:, 0:hw])
                        nc.vector.tensor_copy(gxn[:, dd, hw:W], nj[:, hw:W])
                else:
                    # All four chunk loads are contiguous (1KB/partition);
                    # time-reversal is applied via negative-stride SBUF views
                    # on the matmul rhs instead of per-row DMA descriptors.
                    h0f_v, h0b_v = h0f[:], h0b[:]
                    mir = ds((NCH - 1 - i) * C, C)
                    ff = rhsp.tile([H, C, BL], BF16, tag="ff")
                    nc.sync.dma_start(ff[:], h0f_v[:, ds(i * C, C), :])
                    bm = rhsp.tile([H, C, BL], BF16, tag="bm")
                    nc.sync.dma_start(bm[:], h0b_v[:, mir, :])
                    fm = rhsp.tile([H, C, BL], BF16, tag="fm")
                    nc.sync.dma_start(fm[:], h0f_v[:, mir, :])
                    bb = rhsp.tile([H, C, BL], BF16, tag="bb")
                    nc.sync.dma_start(bb[:], h0b_v[:, ds(i * C, C), :])
                    brv = bm[:, ::-1, :]
                    frv = fm[:, ::-1, :]
                    for dd, (rA, rB) in enumerate(((ff, brv), (frv, bb))):
                        base = dd * 2 * 3 * H
                        for g in range(2):
                            dst = rz_full(2 * g + dd)
                            nc.tensor.matmul(dst, wih1_sb[:, base + g * H: base + (g + 1) * H],
                                             rA[:], start=rz_start(2 * g + dd), stop=False,
                                             skip_group_check=True)
                            nc.tensor.matmul(dst, wih1_sb[:, base + 3 * H + g * H: base + 3 * H + (g + 1) * H],
                                             rB[:], start=False, stop=False, skip_group_check=True)
                            nc.tensor.matmul(dst, bias1_sb[:, dd * 3 * H + g * H: dd * 3 * H + (g + 1) * H],
                                             ones_sb[:], start=False, stop=False, skip_group_check=True)
                        nj = psnjp.tile([H, W], F32, tag="nj")
                        nc.tensor.matmul(nj[:], wih1_sb[:, base + 2 * H: base + 3 * H],
                                         rA[:], start=True, stop=False, skip_group_check=True)
                        nc.tensor.matmul(nj[:], wih1_sb[:, base + 3 * H + 2 * H: base + 3 * H + 3 * H],
                                         rB[:], start=False, stop=False, skip_group_check=True)
                        nc.tensor.matmul(nj[:], bias1_sb[:, dd * 3 * H + 2 * H: dd * 3 * H + 3 * H],
                                         ones_sb[:], start=False, stop=True, skip_group_check=True)
                        # both halves on DVE: scalar-engine copies evict the
                        # sigmoid/tanh activation tables (1.3us reload each)
                        hw = W // 2
                        nc.vector.tensor_copy(gxn[:, dd, 0:hw], nj[:, 0:hw])
                        nc.vector.tensor_copy(gxn[:, dd, hw:W], nj[:, hw:W])

                if STEP_MODE != "nostep":
                    for j in range(C):
                        emit_step(l, j, ring, gxn, rz_ps)
                else:
                    nc.vector.memset(ring[:], 0.0)

                nc.vector.tensor_copy(hstate[:], ring[:, :, C - 1, :])
                nc.sync.dma_start(h_f_dst[:][:, ds(i * C, C), :], ring[:, 0])
                nc.sync.dma_start(h_b_dst[:][:, ds(i * C, C), :], ring[:, 1])

        emit_layer(0)
        if not SKIP_L1:
            emit_layer(1)
        else:
            h1f, h1b = h0f, h0b
        rec.close()

        # ---- head: logits = wout_f . f1[s] + wout_b . b1[s] + bout ----
        if not SKIP_HEAD:
            with tc.tile_pool(name="headp", bufs=3) as hp, \
                 tc.tile_pool(name="headps", bufs=2, space="PSUM") as hps:
                for k in range(NCH):
                    fch = hp.tile([H, W], BF16, tag="fch")
                    nc.sync.dma_start(fch[:], h1f[:][:, k * C:(k + 1) * C, :])
                    bch = hp.tile([H, C, BL], BF16, tag="bch")
                    mk = NCH - 1 - k
                    nc.sync.dma_start(bch[:], h1b[:][:, mk * C:(mk + 1) * C, :])
                    pso = hps.tile([1, W], F32, tag="pso")
                    nc.tensor.matmul(pso[:], wout_sb[:, 0:1], fch[:],
                                     start=True, stop=False, skip_group_check=True)
                    nc.tensor.matmul(pso[:], wout_sb[:, 1:2], bch[:, ::-1, :],
                                     start=False, stop=True, skip_group_check=True)
                    osb = hp.tile([1, W], F32, tag="osb")
                    nc.scalar.activation(osb[:], pso[:], AF.Identity,
                                         bias=bout_sb[0:1, 0:1])
                    nc.sync.dma_start(out_flat[k * W:(k + 1) * W], osb[:])
        stack.close()

    nc.compile()
    return nc


_PROGRAM_CACHE = {}


def _get_program(S=4096, C=64):
    key = (S, C)
    if key not in _PROGRAM_CACHE:
        _PROGRAM_CACHE[key] = build_program(S, C)
    return _PROGRAM_CACHE[key]


def _pack_host_inputs(inputs, S=4096, C=64):
    """Build the per-core input maps from the full problem inputs."""
    W = C * BL
    x = np.asarray(inputs["x"], np.float32)

    # z-gate (g==1) weights and biases are negated so the kernel's sigmoid
    # yields zbar = 1 - z directly (h' = zbar*n + (h - zbar*h)).
    def gsign(g):
        return -1.0 if g == 1 else 1.0

    def gT(w, g):  # transposed gate block: [in, H]
        return np.ascontiguousarray(
            gsign(g) * np.asarray(w, np.float32)[g * H:(g + 1) * H].T)

    whhT = np.stack([
        gT(inputs[f"whh{l}{d}"], g)
        for l in range(2) for d in "fb" for g in range(3)
    ])  # [12,H,H]

    wih0T = np.zeros((2, DIN + 1, 3 * H), np.float32)
    bhhn2 = np.zeros((2, 2, H), np.float32)
    for di, d in enumerate("fb"):
        wih = np.asarray(inputs[f"wih0{d}"], np.float32)  # [3H, DIN]
        bih = np.asarray(inputs[f"bih0{d}"], np.float32)
        bhh = np.asarray(inputs[f"bhh0{d}"], np.float32)
        wih0T[di, :DIN] = wih.T
        for g in range(3):
            bias = bih[g * H:(g + 1) * H].copy()
            if g < 2:
                bias += bhh[g * H:(g + 1) * H]
            wih0T[di, DIN, g * H:(g + 1) * H] = gsign(g) * bias
        wih0T[di, :DIN, H:2 * H] *= -1.0
        bhhn2[0, di] = bhh[2 * H:]

    wih1T = np.zeros((2, 2, H, 3 * H), np.float32)
    bias1 = np.zeros((2, 3 * H), np.float32)
    for di, d in enumerate("fb"):
        wih = np.asarray(inputs[f"wih1{d}"], np.float32)  # [3H, 2H]
        bih = np.asarray(inputs[f"bih1{d}"], np.float32)
        bhh = np.asarray(inputs[f"bhh1{d}"], np.float32)
        for k in range(2):
            for g in range(3):
                wih1T[di, k, :, g * H:(g + 1) * H] = (
                    gsign(g) * wih[g * H:(g + 1) * H, k * H:(k + 1) * H].T)
        for g in range(3):
            bias = bih[g * H:(g + 1) * H].copy()
            if g < 2:
                bias += bhh[g * H:(g + 1) * H]
            bias1[di, g * H:(g + 1) * H] = gsign(g) * bias
        bhhn2[1, di] = bhh[2 * H:]

    sel2 = np.zeros((2, 2 * BL), np.float32)
    sel2[0, :BL] = 1.0
    sel2[1, BL:] = 1.0
    woutp = np.zeros((H, 2), np.float32)
    wout = np.asarray(inputs["wout"], np.float32)
    woutp[:, 0] = wout[0, :H]
    woutp[:, 1] = wout[0, H:]
    boutp = np.asarray(inputs["bout"], np.float32).reshape(1, 1)
    ones = np.ones((1, W), np.float32)

    bf = lambda a: np.ascontiguousarray(a.astype(NP_BF16))
    shared = dict(whhT=bf(whhT), wih0T=bf(wih0T), wih1T=bf(wih1T),
                  bias1=bf(bias1), bhhn2=bf(bhhn2), sel2=bf(sel2),
                  woutp=bf(woutp), boutp=boutp, ones=bf(ones))

    in_maps = []
    for c in range(NCORES):
        xc = x[c * BL:(c + 1) * BL]  # [BL, S, DIN]
        arr = np.ones((DIN + 1, S, BL), np.float32)
        arr[:DIN] = xc.transpose(2, 1, 0)
        xfm = bf(arr.reshape(DIN + 1, S * BL))
        xrm = bf(np.ascontiguousarray(arr[:, ::-1, :].reshape(DIN + 1, S * BL)))
        in_maps.append(dict(shared, xf=xfm, xr=xrm))
    return in_maps


def kernel(**inputs) -> np.ndarray:
    from concourse import bass_utils
    S, C = 4096, 64
    nc = _get_program(S, C)
    in_maps = _pack_host_inputs(inputs, S, C)
    res = bass_utils.run_bass_kernel_spmd(nc, in_maps, core_ids=list(range(NCORES)))
    outs = [r["out"] for r in res.results]  # each [S, BL]
    return np.concatenate([o.T for o in outs], axis=0).astype(np.float32)


# revision 16
# speedup vs baseline: 2.4474x; 1.0217x over previous
"""Trainium2 Bass kernel for a 2-layer bidirectional GRU + linear head.

Problem: B=64, S=4096, D_IN=7, H=128, PyTorch gate order (r, z, n).
Sharding: data-parallel over batch across 8 NeuronCores (8 rows each).

Per-core design (all layouts keep H=128 on the SBUF partition axis):
  - The sequence is processed in chunks of C=64 steps. For each chunk the
    input-gate projections gx = W_ih @ x (+ biases) for the r,z gates of both
    directions are computed by bulk matmuls directly into a 4-bank PSUM tile
    [128, 4, C*8]; the per-step recurrent matmuls (W_hh @ h) then accumulate
    onto their 8-column slice (start=False), so sigmoid reads (xr+hr, xz+hz)
    straight out of PSUM with zero staging ops.
  - The n-gate projections go to an SBUF ring (xn must not receive W_hh@h
    before the r* multiply); b_hh_n is staged into a small PSUM tile with a
    rank-2 matmul, and W_hh_n@h accumulates there.
  - Both directions are packed into the free dim of every elementwise op
    (columns 0:8 forward, 8:16 backward); the backward direction consumes a
    host-reversed copy of x so all its tensors are in scan order ("u" order),
    and the reversal is applied via negative-stride APs when layer 1 / the
    head need time-aligned pairs.
  - The hidden-state ring [128, C, 16] doubles as the output buffer: the
    final h' add of each step writes the ring slot, which the next step's
    matmuls read as rhs and which is DMA'd to DRAM per chunk.
  - All matmul operands are bf16 (PSUM accumulation and every elementwise op
    stay fp32): fp32 matmuls are decomposed by HW into TWO ldweights+matmul
    passes (fp32_mode LOW/HIGH) and disable Fast Weight Load, roughly
    doubling PE occupancy, and fp32 PE activity triggers the HAM power
    throttle (50% duty cycle). bf16 halves the PE instruction stream.
"""

import numpy as np
import ml_dtypes

import concourse.bass as bass
import concourse.tile as tile
from concourse import bacc, mybir
from concourse.bass import ds

F32 = mybir.dt.float32
BF16 = mybir.dt.bfloat16
NP_BF16 = ml_dtypes.bfloat16
AF = mybir.ActivationFunctionType

H = 128
DIN = 7
B = 64
NCORES = 8
BL = B // NCORES  # batch rows per core


STEP_MODE = "full"   # "full" | "nochain" (steps read hstate, no serial dep) | "nostep"
ABLATIONS = set()
USE_HINTS = True
SKIP_L1 = False      # emit only layer 0; head reads h0f/h0b
SKIP_HEAD = False    # skip the head phase (out left zero)


def build_program(S=4096, C=64, n_cores=NCORES):
    """Build the per-core Bass program."""
    NCH = S // C
    W = C * BL  # chunk columns (= matmul moving-dim), 512 for C=64
    nc = bacc.Bacc("TRN2", target_bir_lowering=False, debug=False)

    # ---- DRAM I/O ----
    xf = nc.dram_tensor("xf", [DIN + 1, S * BL], BF16, kind="ExternalInput").ap()
    xr = nc.dram_tensor("xr", [DIN + 1, S * BL], BF16, kind="ExternalInput").ap()
    whhT = nc.dram_tensor("whhT", [12, H, H], BF16, kind="ExternalInput").ap()
    wih0T = nc.dram_tensor("wih0T", [2, DIN + 1, 3 * H], BF16, kind="ExternalInput").ap()
    wih1T = nc.dram_tensor("wih1T", [2, 2, H, 3 * H], BF16, kind="ExternalInput").ap()
    bias1 = nc.dram_tensor("bias1", [2, 3 * H], BF16, kind="ExternalInput").ap()
    bhhn2 = nc.dram_tensor("bhhn2", [2, 2, H], BF16, kind="ExternalInput").ap()
    sel2 = nc.dram_tensor("sel2", [2, 2 * BL], BF16, kind="ExternalInput").ap()
    woutp = nc.dram_tensor("woutp", [H, 2], BF16, kind="ExternalInput").ap()
    boutp = nc.dram_tensor("boutp", [1, 1], F32, kind="ExternalInput").ap()
    ones = nc.dram_tensor("ones", [1, W], BF16, kind="ExternalInput").ap()
    out = nc.dram_tensor("out", [S, BL], F32, kind="ExternalOutput").ap()
    out_flat = out.rearrange("s b -> (s b)")

    with tile.TileContext(nc) as tc:
        from contextlib import ExitStack

        stack = ExitStack()
        consts = stack.enter_context(tc.tile_pool(name="consts", bufs=1))

        # ---- persistent SBUF constants (all bf16 matmul operands) ----
        whh_sb = consts.tile([H, 12 * H], BF16)  # (l,d,g) blocks of 128 cols
        for k in range(12):
            nc.sync.dma_start(whh_sb[:, k * H:(k + 1) * H], whhT[k])
        wih0_sb = consts.tile([DIN + 1, 2 * 3 * H], BF16)
        for d in range(2):
            nc.sync.dma_start(wih0_sb[:, d * 3 * H:(d + 1) * 3 * H], wih0T[d])
        wih1_sb = consts.tile([H, 4 * 3 * H], BF16)  # (d,k) blocks of 384 cols
        for d in range(2):
            for k in range(2):
                c0 = (d * 2 + k) * 3 * H
                nc.sync.dma_start(wih1_sb[:, c0:c0 + 3 * H], wih1T[d, k])
        bias1_sb = consts.tile([1, 2 * 3 * H], BF16)
        nc.sync.dma_start(bias1_sb[:], bias1.rearrange("d m -> (d m)"))
        bhhn_sb = consts.tile([2, 2 * H], BF16)  # [dir_row, layer*128+col]
        for l in range(2):
            nc.sync.dma_start(bhhn_sb[:, l * H:(l + 1) * H], bhhn2[l])
        sel2_sb = consts.tile([2, 2 * BL], BF16)
        nc.sync.dma_start(sel2_sb[:], sel2[:])
        wout_sb = consts.tile([H, 2], BF16)
        nc.sync.dma_start(wout_sb[:], woutp[:])
        bout_sb = consts.tile([1, 1], F32)
        nc.sync.dma_start(bout_sb[:], boutp[:])
        ones_sb = consts.tile([1, W], BF16)
        nc.sync.dma_start(ones_sb[:], ones[:])
        hstate = consts.tile([H, 2 * BL], BF16)

        # ---- internal DRAM: layer outputs (backward dir in scan order) ----
        h0f = nc.dram_tensor("h0f", [H, S, BL], BF16, kind="Internal").ap()
        h0b = nc.dram_tensor("h0b", [H, S, BL], BF16, kind="Internal").ap()
        h1f = nc.dram_tensor("h1f", [H, S, BL], BF16, kind="Internal").ap()
        h1b = nc.dram_tensor("h1b", [H, S, BL], BF16, kind="Internal").ap()

        def whh(l, d, g):
            k = (l * 2 + d) * 3 + g
            return whh_sb[:, k * H:(k + 1) * H]

        rec = ExitStack()
        rhsp = rec.enter_context(tc.tile_pool(name="rhsp", bufs=2))
        gxnp = rec.enter_context(tc.tile_pool(name="gxnp", bufs=2))
        ringp = rec.enter_context(tc.tile_pool(name="ringp", bufs=2))
        stepp = rec.enter_context(tc.tile_pool(name="stepp", bufs=3))
        psp = rec.enter_context(tc.tile_pool(name="psp", bufs=1, space="PSUM"))
        psnjp = rec.enter_context(tc.tile_pool(name="psnjp", bufs=2, space="PSUM"))
        psnp = rec.enter_context(tc.tile_pool(name="psnp", bufs=2, space="PSUM"))

        def emit_step(l, j, ring, gxn, rz_ps):
            if j == 0 or STEP_MODE == "nochain":
                hf, hb = hstate[:, 0:BL], hstate[:, BL:2 * BL]
            else:
                hf, hb = ring[:, 0, j - 1, :], ring[:, 1, j - 1, :]
            js = slice(j * BL, (j + 1) * BL)

            def rzd(sl):
                return rz_ps[:, sl, js]
            SIG = AF.Copy if "act_copy" in ABLATIONS else AF.Sigmoid
            TANH = AF.Copy if "act_copy" in ABLATIONS else AF.Tanh
            # r,z gates first (sigmoid waits only on these 4), accumulating
            # onto the prefilled gx slices
            if "no_rzmm" not in ABLATIONS:
                nc.tensor.matmul(rzd(0), whh(l, 0, 0), hf,
                                 start=False, stop=False, skip_group_check=True)
                nc.tensor.matmul(rzd(1), whh(l, 1, 0), hb,
                                 start=False, stop=False, skip_group_check=True)
                nc.tensor.matmul(rzd(2), whh(l, 0, 1), hf,
                                 start=False, stop=False, skip_group_check=True)
                nc.tensor.matmul(rzd(3), whh(l, 1, 1), hb,
                                 start=False, stop=(j == C - 1), skip_group_check=True)
            # hn = b_hh_n + W_hh_n @ h  (both dirs) in small psum
            psn = psnp.tile([H, 2 * BL], F32, tag="psn")
            nc.tensor.matmul(psn[:], bhhn_sb[:, l * H:(l + 1) * H], sel2_sb[:],
                             start=True, stop=False, skip_group_check=True)
            if "no_nmm" not in ABLATIONS:
                nc.tensor.matmul(psn[:, 0:BL], whh(l, 0, 2), hf,
                                 start=False, stop=False, skip_group_check=True)
                nc.tensor.matmul(psn[:, BL:2 * BL], whh(l, 1, 2), hb,
                                 start=False, stop=True, skip_group_check=True)
            # rz slices: 0:2 = r (f,b); 2:4 = zbar = 1-z (z weights negated on
            # host, so sigmoid yields zbar directly).
            rz = stepp.tile([H, 4, BL], F32, tag="rz")
            nc.scalar.activation(rz[:], rz_ps[:, :, js], SIG)
            h_prev = (hstate[:, :] if (j == 0 or STEP_MODE == "nochain")
                      else ring[:, :, j - 1, :])
            if "no_rn" not in ABLATIONS:
                rn = stepp.tile([H, 2 * BL], F32, tag="rn")
                nc.vector.tensor_mul(rn[:], rz[:, 0:2, :], psn[:])
                arg = stepp.tile([H, 2 * BL], F32, tag="arg")
                nc.vector.tensor_add(arg[:], rn[:], gxn[:, :, js])
                tanh_in = arg
            else:
                tanh_in = None
            # nt2 = (zbar - 1)*h = zbar*h - h runs on DVE during the tanh;
            # h' = zbar*n + h - zbar*h = nz - nt2
            nt2 = stepp.tile([H, 2 * BL], F32, tag="nt2")
            nc.vector.scalar_tensor_tensor(
                nt2[:], rz[:, 2:4, :], 1.0, h_prev,
                op0=mybir.AluOpType.subtract, op1=mybir.AluOpType.mult)
            n_t = stepp.tile([H, 2 * BL], F32, tag="n")
            if tanh_in is not None:
                nc.scalar.activation(n_t[:], tanh_in[:], TANH)
            else:
                nc.scalar.activation(n_t[:], gxn[:, :, js], TANH)
            if "no_upd" not in ABLATIONS:
                nz = stepp.tile([H, 2 * BL], F32, tag="nz")
                nc.vector.tensor_mul(nz[:], rz[:, 2:4, :], n_t[:])
                nc.vector.tensor_sub(ring[:, :, j, :], nz[:], nt2[:])
            else:
                nc.vector.tensor_copy(ring[:, :, j, :], n_t[:])

        def emit_layer(l):
            nc.vector.memset(hstate[:], 0.0)
            h_f_dst, h_b_dst = (h0f, h0b) if l == 0 else (h1f, h1b)
            hints = (mybir.EngineType.PE, mybir.EngineType.DVE) if USE_HINTS else ()
            with tc.For_i(0, NCH, 1, name=f"layer{l}", hint_engines=hints,
                          staggered_reset=True) as i:
                rz_ps = psp.tile([H, 4, W], F32, tag="rzps")
                gxn = gxnp.tile([H, 2, W], F32, tag="gxn")
                ring = ringp.tile([H, 2, C, BL], BF16, tag="ring")
                # start=True clears the whole 2KB PSUM bank, so it may only be
                # used by the first matmul that touches each bank of rz_ps.
                seen_banks = set()

                def rz_start(sl):
                    bank = sl * W // 512
                    if bank in seen_banks:
                        return False
                    seen_banks.add(bank)
                    return True

                def rz_full(sl):
                    return rz_ps[:, sl, :]

                if l == 0:
                    xf_ch = rhsp.tile([DIN + 1, W], BF16, tag="xf")
                    nc.sync.dma_start(xf_ch[:], xf[:, ds(i * W, W)])
                    xr_ch = rhsp.tile([DIN + 1, W], BF16, tag="xr")
                    nc.sync.dma_start(xr_ch[:], xr[:, ds(i * W, W)])
                    srcs = (xf_ch, xr_ch)
                    for dd, src in enumerate(srcs):
                        for g in range(2):  # r, z -> psum
                            nc.tensor.matmul(
                                rz_full(2 * g + dd),
                                wih0_sb[:, dd * 3 * H + g * H: dd * 3 * H + (g + 1) * H],
                                src[:], start=rz_start(2 * g + dd), stop=False,
                                skip_group_check=True)
                        nj = psnjp.tile([H, W], F32, tag="nj")
                        nc.tensor.matmul(
                            nj[:],
                            wih0_sb[:, dd * 3 * H + 2 * H: dd * 3 * H + 3 * H],
                            src[:], start=True, stop=True, skip_group_check=True)
                        # psum -> sbuf n-ring, split across DVE and ACT
                        # both halves on DVE: scalar-engine copies evict the
                        # sigmoid/tanh activation tables (1.3us reload each)
                        hw = W // 2
                        nc.vector.tensor_copy(gxn[:, dd, 0:hw], nj[:, 0:hw])
                        nc.vector.tensor_copy(gxn[:, dd, hw:W], nj[:, hw:W])
                else:
                    # All four chunk loads are contiguous (1KB/partition);
                    # time-reversal is applied via negative-stride SBUF views
                    # on the matmul rhs instead of per-row DMA descriptors.
                    h0f_v, h0b_v = h0f[:], h0b[:]
                    mir = ds((NCH - 1 - i) * C, C)
                    ff = rhsp.tile([H, C, BL], BF16, tag="ff")
                    nc.sync.dma_start(ff[:], h0f_v[:, ds(i * C, C), :])
                    bm = rhsp.tile([H, C, BL], BF16, tag="bm")
                    nc.sync.dma_start(bm[:], h0b_v[:, mir, :])
                    fm = rhsp.tile([H, C, BL], BF16, tag="fm")
                    nc.sync.dma_start(fm[:], h0f_v[:, mir, :])
                    bb = rhsp.tile([H, C, BL], BF16, tag="bb")
                    nc.sync.dma_start(bb[:], h0b_v[:, ds(i * C, C), :])
                    brv = bm[:, ::-1, :]
                    frv = fm[:, ::-1, :]
                    for dd, (rA, rB) in enumerate(((ff, brv), (frv, bb))):
                        base = dd * 2 * 3 * H
                        for g in range(2):
                            dst = rz_full(2 * g + dd)
                            nc.tensor.matmul(dst, wih1_sb[:, base + g * H: base + (g + 1) * H],
                                             rA[:], start=rz_start(2 * g + dd), stop=False,
                                             skip_group_check=True)
                            nc.tensor.matmul(dst, wih1_sb[:, base + 3 * H + g * H: base + 3 * H + (g + 1) * H],
                                             rB[:], start=False, stop=False, skip_group_check=True)
                            nc.tensor.matmul(dst, bias1_sb[:, dd * 3 * H + g * H: dd * 3 * H + (g + 1) * H],
                                             ones_sb[:], start=False, stop=False, skip_group_check=True)
                        nj = psnjp.tile([H, W], F32, tag="nj")
                        nc.tensor.matmul(nj[:], wih1_sb[:, base + 2 * H: base + 3 * H],
                                         rA[:], start=True, stop=False, skip_group_check=True)
                        nc.tensor.matmul(nj[:], wih1_sb[:, base + 3 * H + 2 * H: base + 3 * H + 3 * H],
                                         rB[:], start=False, stop=False, skip_group_check=True)
                        nc.tensor.matmul(nj[:], bias1_sb[:, dd * 3 * H + 2 * H: dd * 3 * H + 3 * H],
                                         ones_sb[:], start=False, stop=True, skip_group_check=True)
                        # both halves on DVE: scalar-engine copies evict the
                        # sigmoid/tanh activation tables (1.3us reload each)
                        hw = W // 2
                        nc.vector.tensor_copy(gxn[:, dd, 0:hw], nj[:, 0:hw])
                        nc.vector.tensor_copy(gxn[:, dd, hw:W], nj[:, hw:W])

                if STEP_MODE != "nostep":
                    for j in range(C):
                        emit_step(l, j, ring, gxn, rz_ps)
                else:
                    nc.vector.memset(ring[:], 0.0)

                nc.vector.tensor_copy(hstate[:], ring[:, :, C - 1, :])
                nc.sync.dma_start(h_f_dst[:][:, ds(i * C, C), :], ring[:, 0])
                nc.sync.dma_start(h_b_dst[:][:, ds(i * C, C), :], ring[:, 1])

        emit_layer(0)
        if not SKIP_L1:
            emit_layer(1)
        else:
            h1f, h1b = h0f, h0b
        rec.close()

        # ---- head: logits = wout_f . f1[s] + wout_b . b1[s] + bout ----
        if not SKIP_HEAD:
            with tc.tile_pool(name="headp", bufs=3) as hp, \
                 tc.tile_pool(name="headps", bufs=2, space="PSUM") as hps:
                for k in range(NCH):
                    fch = hp.tile([H, W], BF16, tag="fch")
                    nc.sync.dma_start(fch[:], h1f[:][:, k * C:(k + 1) * C, :])
                    bch = hp.tile([H, C, BL], BF16, tag="bch")
                    mk = NCH - 1 - k
                    nc.sync.dma_start(bch[:], h1b[:][:, mk * C:(mk + 1) * C, :])
                    pso = hps.tile([1, W], F32, tag="pso")
                    nc.tensor.matmul(pso[:], wout_sb[:, 0:1], fch[:],
                                     start=True, stop=False, skip_group_check=True)
                    nc.tensor.matmul(pso[:], wout_sb[:, 1:2], bch[:, ::-1, :],
                                     start=False, stop=True, skip_group_check=True)
                    osb = hp.tile([1, W], F32, tag="osb")
                    nc.scalar.activation(osb[:], pso[:], AF.Identity,
                                         bias=bout_sb[0:1, 0:1])
                    nc.sync.dma_start(out_flat[k * W:(k + 1) * W], osb[:])
        stack.close()

    nc.compile()
    return nc


_PROGRAM_CACHE = {}


def _get_program(S=4096, C=64):
    key = (S, C)
    if key not in _PROGRAM_CACHE:
        _PROGRAM_CACHE[key] = build_program(S, C)
    return _PROGRAM_CACHE[key]


def _pack_host_inputs(inputs, S=4096, C=64):
    """Build the per-core input maps from the full problem inputs."""
    W = C * BL
    x = np.asarray(inputs["x"], np.float32)

    # z-gate (g==1) weights and biases are negated so the kernel's sigmoid
    # yields zbar = 1 - z directly (h' = zbar*n + (h - zbar*h)).
    def gsign(g):
        return -1.0 if g == 1 else 1.0

    def gT(w, g):  # transposed gate block: [in, H]
        return np.ascontiguousarray(
            gsign(g) * np.asarray(w, np.float32)[g * H:(g + 1) * H].T)

    whhT = np.stack([
        gT(inputs[f"whh{l}{d}"], g)
        for l in range(2) for d in "fb" for g in range(3)
    ])  # [12,H,H]

    wih0T = np.zeros((2, DIN + 1, 3 * H), np.float32)
    bhhn2 = np.zeros((2, 2, H), np.float32)
    for di, d in enumerate("fb"):
        wih = np.asarray(inputs[f"wih0{d}"], np.float32)  # [3H, DIN]
        bih = np.asarray(inputs[f"bih0{d}"], np.float32)
        bhh = np.asarray(inputs[f"bhh0{d}"], np.float32)
        wih0T[di, :DIN] = wih.T
        for g in range(3):
            bias = bih[g * H:(g + 1) * H].copy()
            if g < 2:
                bias += bhh[g * H:(g + 1) * H]
            wih0T[di, DIN, g * H:(g + 1) * H] = gsign(g) * bias
        wih0T[di, :DIN, H:2 * H] *= -1.0
        bhhn2[0, di] = bhh[2 * H:]

    wih1T = np.zeros((2, 2, H, 3 * H), np.float32)
    bias1 = np.zeros((2, 3 * H), np.float32)
    for di, d in enumerate("fb"):
        wih = np.asarray(inputs[f"wih1{d}"], np.float32)  # [3H, 2H]
        bih = np.asarray(inputs[f"bih1{d}"], np.float32)
        bhh = np.asarray(inputs[f"bhh1{d}"], np.float32)
        for k in range(2):
            for g in range(3):
                wih1T[di, k, :, g * H:(g + 1) * H] = (
                    gsign(g) * wih[g * H:(g + 1) * H, k * H:(k + 1) * H].T)
        for g in range(3):
            bias = bih[g * H:(g + 1) * H].copy()
            if g < 2:
                bias += bhh[g * H:(g + 1) * H]
            bias1[di, g * H:(g + 1) * H] = gsign(g) * bias
        bhhn2[1, di] = bhh[2 * H:]

    sel2 = np.zeros((2, 2 * BL), np.float32)
    sel2[0, :BL] = 1.0
    sel2[1, BL:] = 1.0
    woutp = np.zeros((H, 2), np.float32)
    wout = np.asarray(inputs["wout"], np.float32)
    woutp[:, 0] = wout[0, :H]
    woutp[:, 1] = wout[0, H:]
    boutp = np.asarray(inputs["bout"], np.float32).reshape(1, 1)
    ones = np.ones((1, W), np.float32)

    bf = lambda a: np.ascontiguousarray(a.astype(NP_BF16))
    shared = dict(whhT=bf(whhT), wih0T=bf(wih0T), wih1T=bf(wih1T),
                  bias1=bf(bias1), bhhn2=bf(bhhn2), sel2=bf(sel2),
                  woutp=bf(woutp), boutp=boutp, ones=bf(ones))

    in_maps = []
    for c in range(NCORES):
        xc = x[c * BL:(c + 1) * BL]  # [BL, S, DIN]
        arr = np.ones((DIN + 1, S, BL), np.float32)
        arr[:DIN] = xc.transpose(2, 1, 0)
        xfm = bf(arr.reshape(DIN + 1, S * BL))
        xrm = bf(np.ascontiguousarray(arr[:, ::-1, :].reshape(DIN + 1, S * BL)))
        in_maps.append(dict(shared, xf=xfm, xr=xrm))
    return in_maps


def kernel(**inputs) -> np.ndarray:
    from concourse import bass_utils
    S, C = 4096, 64
    nc = _get_program(S, C)
    in_maps = _pack_host_inputs(inputs, S, C)
    res = bass_utils.run_bass_kernel_spmd(nc, in_maps, core_ids=list(range(NCORES)))
    outs = [r["out"] for r in res.results]  # each [S, BL]
    return np.concatenate([o.T for o in outs], axis=0).astype(np.float32)


# revision 18
# speedup vs baseline: 2.4685x; 1.0086x over previous
"""Trainium2 Bass kernel for a 2-layer bidirectional GRU + linear head.

Problem: B=64, S=4096, D_IN=7, H=128, PyTorch gate order (r, z, n).
Sharding: data-parallel over batch across 8 NeuronCores (8 rows each).

Per-core design (all layouts keep H=128 on the SBUF partition axis):
  - The sequence is processed in chunks of C=64 steps. For each chunk the
    input-gate projections gx = W_ih @ x (+ biases) for the r,z gates of both
    directions are computed by bulk matmuls directly into a 4-bank PSUM tile
    [128, 4, C*8]; the per-step recurrent matmuls (W_hh @ h) then accumulate
    onto their 8-column slice (start=False), so sigmoid reads (xr+hr, xz+hz)
    straight out of PSUM with zero staging ops.
  - The n-gate projections go to an SBUF ring (xn must not receive W_hh@h
    before the r* multiply); b_hh_n is staged into a small PSUM tile with a
    rank-2 matmul, and W_hh_n@h accumulates there.
  - Both directions are packed into the free dim of every elementwise op
    (columns 0:8 forward, 8:16 backward); the backward direction consumes a
    host-reversed copy of x so all its tensors are in scan order ("u" order),
    and the reversal is applied via negative-stride APs when layer 1 / the
    head need time-aligned pairs.
  - The hidden-state ring [128, C, 16] doubles as the output buffer: the
    final h' add of each step writes the ring slot, which the next step's
    matmuls read as rhs and which is DMA'd to DRAM per chunk.
  - All matmul operands are bf16 (PSUM accumulation and every elementwise op
    stay fp32): fp32 matmuls are decomposed by HW into TWO ldweights+matmul
    passes (fp32_mode LOW/HIGH) and disable Fast Weight Load, roughly
    doubling PE occupancy, and fp32 PE activity triggers the HAM power
    throttle (50% duty cycle). bf16 halves the PE instruction stream.
"""

import numpy as np
import ml_dtypes

import concourse.bass as bass
import concourse.tile as tile
from concourse import bacc, mybir
from concourse.bass import ds

F32 = mybir.dt.float32
BF16 = mybir.dt.bfloat16
NP_BF16 = ml_dtypes.bfloat16
AF = mybir.ActivationFunctionType

H = 128
DIN = 7
B = 64
NCORES = 8
BL = B // NCORES  # batch rows per core


STEP_MODE = "full"   # "full" | "nochain" (steps read hstate, no serial dep) | "nostep"
ABLATIONS = set()
USE_HINTS = True
SKIP_L1 = False      # emit only layer 0; head reads h0f/h0b
SKIP_HEAD = False    # skip the head phase (out left zero)


def build_program(S=4096, C=64, n_cores=NCORES):
    """Build the per-core Bass program."""
    NCH = S // C
    W = C * BL  # chunk columns (= matmul moving-dim), 512 for C=64
    nc = bacc.Bacc("TRN2", target_bir_lowering=False, debug=False)

    # ---- DRAM I/O ----
    xf = nc.dram_tensor("xf", [DIN + 1, S * BL], BF16, kind="ExternalInput").ap()
    xr = nc.dram_tensor("xr", [DIN + 1, S * BL], BF16, kind="ExternalInput").ap()
    whhT = nc.dram_tensor("whhT", [12, H, H], BF16, kind="ExternalInput").ap()
    wih0T = nc.dram_tensor("wih0T", [2, DIN + 1, 3 * H], BF16, kind="ExternalInput").ap()
    wih1T = nc.dram_tensor("wih1T", [2, 2, H, 3 * H], BF16, kind="ExternalInput").ap()
    bias1 = nc.dram_tensor("bias1", [2, 3 * H], BF16, kind="ExternalInput").ap()
    bhhn2 = nc.dram_tensor("bhhn2", [2, 2, H], BF16, kind="ExternalInput").ap()
    sel2 = nc.dram_tensor("sel2", [2, 2 * BL], BF16, kind="ExternalInput").ap()
    woutp = nc.dram_tensor("woutp", [H, 2], BF16, kind="ExternalInput").ap()
    boutp = nc.dram_tensor("boutp", [1, 1], F32, kind="ExternalInput").ap()
    ones = nc.dram_tensor("ones", [1, W], BF16, kind="ExternalInput").ap()
    out = nc.dram_tensor("out", [S, BL], F32, kind="ExternalOutput").ap()
    out_flat = out.rearrange("s b -> (s b)")

    with tile.TileContext(nc) as tc:
        from contextlib import ExitStack

        stack = ExitStack()
        consts = stack.enter_context(tc.tile_pool(name="consts", bufs=1))

        # ---- persistent SBUF constants (all bf16 matmul operands) ----
        whh_sb = consts.tile([H, 12 * H], BF16)  # (l,d,g) blocks of 128 cols
        for k in range(12):
            nc.sync.dma_start(whh_sb[:, k * H:(k + 1) * H], whhT[k])
        wih0_sb = consts.tile([DIN + 1, 2 * 3 * H], BF16)
        for d in range(2):
            nc.sync.dma_start(wih0_sb[:, d * 3 * H:(d + 1) * 3 * H], wih0T[d])
        wih1_sb = consts.tile([H, 4 * 3 * H], BF16)  # (d,k) blocks of 384 cols
        for d in range(2):
            for k in range(2):
                c0 = (d * 2 + k) * 3 * H
                nc.sync.dma_start(wih1_sb[:, c0:c0 + 3 * H], wih1T[d, k])
        bias1_sb = consts.tile([1, 2 * 3 * H], BF16)
        nc.sync.dma_start(bias1_sb[:], bias1.rearrange("d m -> (d m)"))
        bhhn_sb = consts.tile([2, 2 * H], BF16)  # [dir_row, layer*128+col]
        for l in range(2):
            nc.sync.dma_start(bhhn_sb[:, l * H:(l + 1) * H], bhhn2[l])
        sel2_sb = consts.tile([2, 2 * BL], BF16)
        nc.sync.dma_start(sel2_sb[:], sel2[:])
        wout_sb = consts.tile([H, 2], BF16)
        nc.sync.dma_start(wout_sb[:], woutp[:])
        bout_sb = consts.tile([1, 1], F32)
        nc.sync.dma_start(bout_sb[:], boutp[:])
        ones_sb = consts.tile([1, W], BF16)
        nc.sync.dma_start(ones_sb[:], ones[:])
        hstate = consts.tile([H, 2 * BL], BF16)

        # ---- internal DRAM: layer outputs (backward dir in scan order) ----
        h0f = nc.dram_tensor("h0f", [H, S, BL], BF16, kind="Internal").ap()
        h0b = nc.dram_tensor("h0b", [H, S, BL], BF16, kind="Internal").ap()
        h1f = nc.dram_tensor("h1f", [H, S, BL], BF16, kind="Internal").ap()
        h1b = nc.dram_tensor("h1b", [H, S, BL], BF16, kind="Internal").ap()

        def whh(l, d, g):
            k = (l * 2 + d) * 3 + g
            return whh_sb[:, k * H:(k + 1) * H]

        rec = ExitStack()
        rhsp = rec.enter_context(tc.tile_pool(name="rhsp", bufs=2))
        gxnp = rec.enter_context(tc.tile_pool(name="gxnp", bufs=2))
        ringp = rec.enter_context(tc.tile_pool(name="ringp", bufs=2))
        stepp = rec.enter_context(tc.tile_pool(name="stepp", bufs=3))
        psp = rec.enter_context(tc.tile_pool(name="psp", bufs=1, space="PSUM"))
        psnjp = rec.enter_context(tc.tile_pool(name="psnjp", bufs=2, space="PSUM"))
        psnp = rec.enter_context(tc.tile_pool(name="psnp", bufs=2, space="PSUM"))

        def emit_step(l, j, ring, gxn, rz_ps):
            if j == 0 or STEP_MODE == "nochain":
                hf, hb = hstate[:, 0:BL], hstate[:, BL:2 * BL]
            else:
                hf, hb = ring[:, 0, j - 1, :], ring[:, 1, j - 1, :]
            js = slice(j * BL, (j + 1) * BL)

            def rzd(sl):
                return rz_ps[:, sl, js]
            SIG = AF.Copy if "act_copy" in ABLATIONS else AF.Sigmoid
            TANH = AF.Copy if "act_copy" in ABLATIONS else AF.Tanh
            # r,z gates first (sigmoid waits only on these 4), accumulating
            # onto the prefilled gx slices
            if "no_rzmm" not in ABLATIONS:
                nc.tensor.matmul(rzd(0), whh(l, 0, 0), hf,
                                 start=False, stop=False, skip_group_check=True)
                nc.tensor.matmul(rzd(1), whh(l, 1, 0), hb,
                                 start=False, stop=False, skip_group_check=True)
                nc.tensor.matmul(rzd(2), whh(l, 0, 1), hf,
                                 start=False, stop=False, skip_group_check=True)
                nc.tensor.matmul(rzd(3), whh(l, 1, 1), hb,
                                 start=False, stop=(j == C - 1), skip_group_check=True)
            # hn = b_hh_n + W_hh_n @ h  (both dirs) in small psum
            psn = psnp.tile([H, 2 * BL], F32, tag="psn")
            nc.tensor.matmul(psn[:], bhhn_sb[:, l * H:(l + 1) * H], sel2_sb[:],
                             start=True, stop=False, skip_group_check=True)
            if "no_nmm" not in ABLATIONS:
                nc.tensor.matmul(psn[:, 0:BL], whh(l, 0, 2), hf,
                                 start=False, stop=False, skip_group_check=True)
                nc.tensor.matmul(psn[:, BL:2 * BL], whh(l, 1, 2), hb,
                                 start=False, stop=True, skip_group_check=True)
            # rz slices: 0:2 = r (f,b); 2:4 = zbar = 1-z (z weights negated on
            # host, so sigmoid yields zbar directly). Split into two ACTIVATEs
            # so the r half fires as soon as the two r matmuls land; bf16 out
            # engages the 2x write mode.
            rz = stepp.tile([H, 4, BL], BF16, tag="rz")
            nc.scalar.activation(rz[:, 0:2, :], rz_ps[:, 0:2, js], SIG)
            nc.scalar.activation(rz[:, 2:4, :], rz_ps[:, 2:4, js], SIG)
            h_prev = (hstate[:, :] if (j == 0 or STEP_MODE == "nochain")
                      else ring[:, :, j - 1, :])
            if "no_rn" not in ABLATIONS:
                rn = stepp.tile([H, 2 * BL], F32, tag="rn")
                nc.vector.tensor_mul(rn[:], rz[:, 0:2, :], psn[:])
                arg = stepp.tile([H, 2 * BL], F32, tag="arg")
                nc.vector.tensor_add(arg[:], rn[:], gxn[:, :, js])
                tanh_in = arg
            else:
                tanh_in = None
            # nt2 = (zbar - 1)*h = zbar*h - h runs on DVE during the tanh;
            # h' = zbar*n + h - zbar*h = nz - nt2
            nt2 = stepp.tile([H, 2 * BL], F32, tag="nt2")
            nc.vector.scalar_tensor_tensor(
                nt2[:], rz[:, 2:4, :], 1.0, h_prev,
                op0=mybir.AluOpType.subtract, op1=mybir.AluOpType.mult)
            n_t = stepp.tile([H, 2 * BL], BF16, tag="n")
            if tanh_in is not None:
                nc.scalar.activation(n_t[:], tanh_in[:], TANH)
            else:
                nc.scalar.activation(n_t[:], gxn[:, :, js], TANH)
            if "no_upd" not in ABLATIONS:
                nz = stepp.tile([H, 2 * BL], F32, tag="nz")
                nc.vector.tensor_mul(nz[:], rz[:, 2:4, :], n_t[:])
                nc.vector.tensor_sub(ring[:, :, j, :], nz[:], nt2[:])
            else:
                nc.vector.tensor_copy(ring[:, :, j, :], n_t[:])

        def emit_layer(l):
            nc.vector.memset(hstate[:], 0.0)
            h_f_dst, h_b_dst = (h0f, h0b) if l == 0 else (h1f, h1b)
            hints = (mybir.EngineType.PE, mybir.EngineType.DVE) if USE_HINTS else ()
            with tc.For_i(0, NCH, 1, name=f"layer{l}", hint_engines=hints,
                          staggered_reset=True) as i:
                rz_ps = psp.tile([H, 4, W], F32, tag="rzps")
                gxn = gxnp.tile([H, 2, W], F32, tag="gxn")
                ring = ringp.tile([H, 2, C, BL], BF16, tag="ring")
                # start=True clears the whole 2KB PSUM bank, so it may only be
                # used by the first matmul that touches each bank of rz_ps.
                seen_banks = set()

                def rz_start(sl):
                    bank = sl * W // 512
                    if bank in seen_banks:
                        return False
                    seen_banks.add(bank)
                    return True

                def rz_full(sl):
                    return rz_ps[:, sl, :]

                if l == 0:
                    xf_ch = rhsp.tile([DIN + 1, W], BF16, tag="xf")
                    nc.sync.dma_start(xf_ch[:], xf[:, ds(i * W, W)])
                    xr_ch = rhsp.tile([DIN + 1, W], BF16, tag="xr")
                    nc.sync.dma_start(xr_ch[:], xr[:, ds(i * W, W)])
                    srcs = (xf_ch, xr_ch)
                    for dd, src in enumerate(srcs):
                        for g in range(2):  # r, z -> psum
                            nc.tensor.matmul(
                                rz_full(2 * g + dd),
                                wih0_sb[:, dd * 3 * H + g * H: dd * 3 * H + (g + 1) * H],
                                src[:], start=rz_start(2 * g + dd), stop=False,
                                skip_group_check=True)
                        nj = psnjp.tile([H, W], F32, tag="nj")
                        nc.tensor.matmul(
                            nj[:],
                            wih0_sb[:, dd * 3 * H + 2 * H: dd * 3 * H + 3 * H],
                            src[:], start=True, stop=True, skip_group_check=True)
                        # psum -> sbuf n-ring, split across DVE and ACT
                        # both halves on DVE: scalar-engine copies evict the
                        # sigmoid/tanh activation tables (1.3us reload each)
                        hw = W // 2
                        nc.vector.tensor_copy(gxn[:, dd, 0:hw], nj[:, 0:hw])
                        nc.vector.tensor_copy(gxn[:, dd, hw:W], nj[:, hw:W])
                else:
                    # All four chunk loads are contiguous (1KB/partition);
                    # time-reversal is applied via negative-stride SBUF views
                    # on the matmul rhs instead of per-row DMA descriptors.
                    h0f_v, h0b_v = h0f[:], h0b[:]
                    mir = ds((NCH - 1 - i) * C, C)
                    ff = rhsp.tile([H, C, BL], BF16, tag="ff")
                    nc.sync.dma_start(ff[:], h0f_v[:, ds(i * C, C), :])
                    bm = rhsp.tile([H, C, BL], BF16, tag="bm")
                    nc.sync.dma_start(bm[:], h0b_v[:, mir, :])
                    fm = rhsp.tile([H, C, BL], BF16, tag="fm")
                    nc.sync.dma_start(fm[:], h0f_v[:, mir, :])
                    bb = rhsp.tile([H, C, BL], BF16, tag="bb")
                    nc.sync.dma_start(bb[:], h0b_v[:, ds(i * C, C), :])
                    brv = bm[:, ::-1, :]
                    frv = fm[:, ::-1, :]
                    for dd, (rA, rB) in enumerate(((ff, brv), (frv, bb))):
                        base = dd * 2 * 3 * H
                        for g in range(2):
                            dst = rz_full(2 * g + dd)
                            nc.tensor.matmul(dst, wih1_sb[:, base + g * H: base + (g + 1) * H],
                                             rA[:], start=rz_start(2 * g + dd), stop=False,
                                             skip_group_check=True)
                            nc.tensor.matmul(dst, wih1_sb[:, base + 3 * H + g * H: base + 3 * H + (g + 1) * H],
                                             rB[:], start=False, stop=False, skip_group_check=True)
                            nc.tensor.matmul(dst, bias1_sb[:, dd * 3 * H + g * H: dd * 3 * H + (g + 1) * H],
                                             ones_sb[:], start=False, stop=False, skip_group_check=True)
                        nj = psnjp.tile([H, W], F32, tag="nj")
                        nc.tensor.matmul(nj[:], wih1_sb[:, base + 2 * H: base + 3 * H],
                                         rA[:], start=True, stop=False, skip_group_check=True)
                        nc.tensor.matmul(nj[:], wih1_sb[:, base + 3 * H + 2 * H: base + 3 * H + 3 * H],
                                         rB[:], start=False, stop=False, skip_group_check=True)
                        nc.tensor.matmul(nj[:], bias1_sb[:, dd * 3 * H + 2 * H: dd * 3 * H + 3 * H],
                                         ones_sb[:], start=False, stop=True, skip_group_check=True)
                        # both halves on DVE: scalar-engine copies evict the
                        # sigmoid/tanh activation tables (1.3us reload each)
                        hw = W // 2
                        nc.vector.tensor_copy(gxn[:, dd, 0:hw], nj[:, 0:hw])
                        nc.vector.tensor_copy(gxn[:, dd, hw:W], nj[:, hw:W])

                if STEP_MODE != "nostep":
                    for j in range(C):
                        emit_step(l, j, ring, gxn, rz_ps)
                else:
                    nc.vector.memset(ring[:], 0.0)

                nc.vector.tensor_copy(hstate[:], ring[:, :, C - 1, :])
                nc.sync.dma_start(h_f_dst[:][:, ds(i * C, C), :], ring[:, 0])
                nc.sync.dma_start(h_b_dst[:][:, ds(i * C, C), :], ring[:, 1])

        emit_layer(0)
        if not SKIP_L1:
            emit_layer(1)
        else:
            h1f, h1b = h0f, h0b
        rec.close()

        # ---- head: logits = wout_f . f1[s] + wout_b . b1[s] + bout ----
        if not SKIP_HEAD:
            with tc.tile_pool(name="headp", bufs=3) as hp, \
                 tc.tile_pool(name="headps", bufs=2, space="PSUM") as hps:
                for k in range(NCH):
                    fch = hp.tile([H, W], BF16, tag="fch")
                    nc.sync.dma_start(fch[:], h1f[:][:, k * C:(k + 1) * C, :])
                    bch = hp.tile([H, C, BL], BF16, tag="bch")
                    mk = NCH - 1 - k
                    nc.sync.dma_start(bch[:], h1b[:][:, mk * C:(mk + 1) * C, :])
                    pso = hps.tile([1, W], F32, tag="pso")
                    nc.tensor.matmul(pso[:], wout_sb[:, 0:1], fch[:],
                                     start=True, stop=False, skip_group_check=True)
                    nc.tensor.matmul(pso[:], wout_sb[:, 1:2], bch[:, ::-1, :],
                                     start=False, stop=True, skip_group_check=True)
                    osb = hp.tile([1, W], F32, tag="osb")
                    nc.scalar.activation(osb[:], pso[:], AF.Identity,
                                         bias=bout_sb[0:1, 0:1])
                    nc.sync.dma_start(out_flat[k * W:(k + 1) * W], osb[:])
        stack.close()

    nc.compile()
    return nc


_PROGRAM_CACHE = {}


def _get_program(S=4096, C=64):
    key = (S, C)
    if key not in _PROGRAM_CACHE:
        _PROGRAM_CACHE[key] = build_program(S, C)
    return _PROGRAM_CACHE[key]


def _pack_host_inputs(inputs, S=4096, C=64):
    """Build the per-core input maps from the full problem inputs."""
    W = C * BL
    x = np.asarray(inputs["x"], np.float32)

    # z-gate (g==1) weights and biases are negated so the kernel's sigmoid
    # yields zbar = 1 - z directly (h' = zbar*n + (h - zbar*h)).
    def gsign(g):
        return -1.0 if g == 1 else 1.0

    def gT(w, g):  # transposed gate block: [in, H]
        return np.ascontiguousarray(
            gsign(g) * np.asarray(w, np.float32)[g * H:(g + 1) * H].T)

    whhT = np.stack([
        gT(inputs[f"whh{l}{d}"], g)
        for l in range(2) for d in "fb" for g in range(3)
    ])  # [12,H,H]

    wih0T = np.zeros((2, DIN + 1, 3 * H), np.float32)
    bhhn2 = np.zeros((2, 2, H), np.float32)
    for di, d in enumerate("fb"):
        wih = np.asarray(inputs[f"wih0{d}"], np.float32)  # [3H, DIN]
        bih = np.asarray(inputs[f"bih0{d}"], np.float32)
        bhh = np.asarray(inputs[f"bhh0{d}"], np.float32)
        wih0T[di, :DIN] = wih.T
        for g in range(3):
            bias = bih[g * H:(g + 1) * H].copy()
            if g < 2:
                bias += bhh[g * H:(g + 1) * H]
            wih0T[di, DIN, g * H:(g + 1) * H] = gsign(g) * bias
        wih0T[di, :DIN, H:2 * H] *= -1.0
        bhhn2[0, di] = bhh[2 * H:]

    wih1T = np.zeros((2, 2, H, 3 * H), np.float32)
    bias1 = np.zeros((2, 3 * H), np.float32)
    for di, d in enumerate("fb"):
        wih = np.asarray(inputs[f"wih1{d}"], np.float32)  # [3H, 2H]
        bih = np.asarray(inputs[f"bih1{d}"], np.float32)
        bhh = np.asarray(inputs[f"bhh1{d}"], np.float32)
        for k in range(2):
            for g in range(3):
                wih1T[di, k, :, g * H:(g + 1) * H] = (
                    gsign(g) * wih[g * H:(g + 1) * H, k * H:(k + 1) * H].T)
        for g in range(3):
            bias = bih[g * H:(g + 1) * H].copy()
            if g < 2:
                bias += bhh[g * H:(g + 1) * H]
            bias1[di, g * H:(g + 1) * H] = gsign(g) * bias
        bhhn2[1, di] = bhh[2 * H:]

    sel2 = np.zeros((2, 2 * BL), np.float32)
    sel2[0, :BL] = 1.0
    sel2[1, BL:] = 1.0
    woutp = np.zeros((H, 2), np.float32)
    wout = np.asarray(inputs["wout"], np.float32)
    woutp[:, 0] = wout[0, :H]
    woutp[:, 1] = wout[0, H:]
    boutp = np.asarray(inputs["bout"], np.float32).reshape(1, 1)
    ones = np.ones((1, W), np.float32)

    bf = lambda a: np.ascontiguousarray(a.astype(NP_BF16))
    shared = dict(whhT=bf(whhT), wih0T=bf(wih0T), wih1T=bf(wih1T),
                  bias1=bf(bias1), bhhn2=bf(bhhn2), sel2=bf(sel2),
                  woutp=bf(woutp), boutp=boutp, ones=bf(ones))

    in_maps = []
    for c in range(NCORES):
        xc = x[c * BL:(c + 1) * BL]  # [BL, S, DIN]
        arr = np.ones((DIN + 1, S, BL), np.float32)
        arr[:DIN] = xc.transpose(2, 1, 0)
        xfm = bf(arr.reshape(DIN + 1, S * BL))
        xrm = bf(np.ascontiguousarray(arr[:, ::-1, :].reshape(DIN + 1, S * BL)))
        in_maps.append(dict(shared, xf=xfm, xr=xrm))
    return in_maps


def kernel(**inputs) -> np.ndarray:
    from concourse import bass_utils
    S, C = 4096, 64
    nc = _get_program(S, C)
    in_maps = _pack_host_inputs(inputs, S, C)
    res = bass_utils.run_bass_kernel_spmd(nc, in_maps, core_ids=list(range(NCORES)))
    outs = [r["out"] for r in res.results]  # each [S, BL]
    return np.concatenate([o.T for o in outs], axis=0).astype(np.float32)


# revision 22
# speedup vs baseline: 2.5364x; 1.0275x over previous
"""Trainium2 Bass kernel for a 2-layer bidirectional GRU + linear head.

Problem: B=64, S=4096, D_IN=7, H=128, PyTorch gate order (r, z, n).
Sharding: data-parallel over batch across 8 NeuronCores (8 rows each).

Per-core design (all layouts keep H=128 on the SBUF partition axis):
  - The sequence is processed in chunks of C=64 steps, each split into two
    32-step halves (A/B) with separate PSUM tiles. The input-gate projections
    gx = W_ih @ x (+ biases) for r,z go into 2-bank PSUM tiles per half; the
    per-step recurrent matmuls (W_hh @ h) accumulate onto their 8-column
    slice (start=False), so sigmoid reads (xr+hr, xz+hz) straight out of
    PSUM. The n-gate projections live in their own 1-bank PSUM tile per half
    and are read directly by the DVE (no SBUF staging).
  - Software pipelining across chunks: chunk i+1's input DMAs issue before
    chunk i's steps run (transfers overlap compute), and chunk i+1's A-half
    bulk matmuls are emitted after chunk i's A-half steps, so they execute
    mid-chunk (their PSUM WAR is against step 31's sigmoid, not step 63's).
    DRAM tensors are padded one chunk so the i+1 prefetch never reads OOB.
  - Both directions are packed into the free dim of every elementwise op
    (columns 0:8 forward, 8:16 backward); the backward direction consumes a
    host-reversed copy of x so all its tensors are in scan order, and the
    time-reversal for layer 1 / the head is applied via negative-stride SBUF
    views on the matmul rhs (contiguous DMAs; no per-row descriptors).
  - The hidden-state ring [128, 2, C, BL] doubles as the output buffer and
    is streamed to DRAM in 8-step blocks as the h' writes land, so the next
    chunk's step-0 write never waits on a whole-ring DMA.
  - The GRU update uses zbar = 1-z (z weights negated on host):
    h' = zbar*n - ((zbar-1)*h), with (zbar-1)*h computed by a fused
    scalar_tensor_tensor on the DVE during the tanh.
  - All matmul operands are bf16 (PSUM accumulation and every elementwise op
    stay fp32): fp32 matmuls are decomposed by HW into TWO ldweights+matmul
    passes and disable Fast Weight Load, roughly doubling PE occupancy, and
    fp32 PE activity triggers the HAM power throttle (50% duty cycle).
"""

import numpy as np
import ml_dtypes

import concourse.bass as bass
import concourse.tile as tile
from concourse import bacc, mybir
from concourse.bass import ds

F32 = mybir.dt.float32
BF16 = mybir.dt.bfloat16
NP_BF16 = ml_dtypes.bfloat16
AF = mybir.ActivationFunctionType
ALU = mybir.AluOpType

H = 128
DIN = 7
B = 64
NCORES = 8
BL = B // NCORES  # batch rows per core

STEP_MODE = "full"   # "full" | "nochain" (steps read hstate, no serial dep)
USE_HINTS = True


def build_program(S=4096, C=64, n_cores=NCORES):
    """Build the per-core Bass program."""
    NCH = S // C
    W = C * BL       # chunk columns, 512 for C=64
    C2 = C // 2      # steps per half-chunk
    W2 = C2 * BL     # half-chunk columns, 256
    nc = bacc.Bacc("TRN2", target_bir_lowering=False, debug=False)

    # ---- DRAM I/O (x padded one chunk at the end for the i+1 prefetch) ----
    xf = nc.dram_tensor("xf", [DIN + 1, (S + C) * BL], BF16, kind="ExternalInput").ap()
    xr = nc.dram_tensor("xr", [DIN + 1, (S + C) * BL], BF16, kind="ExternalInput").ap()
    whhT = nc.dram_tensor("whhT", [12, H, H], BF16, kind="ExternalInput").ap()
    wih0T = nc.dram_tensor("wih0T", [2, DIN + 1, 3 * H], BF16, kind="ExternalInput").ap()
    wih1T = nc.dram_tensor("wih1T", [2, 2, H, 3 * H], BF16, kind="ExternalInput").ap()
    bias1 = nc.dram_tensor("bias1", [2, 3 * H], BF16, kind="ExternalInput").ap()
    bhhn2 = nc.dram_tensor("bhhn2", [2, 2, H], BF16, kind="ExternalInput").ap()
    sel2 = nc.dram_tensor("sel2", [2, 2 * BL], BF16, kind="ExternalInput").ap()
    woutp = nc.dram_tensor("woutp", [H, 2], BF16, kind="ExternalInput").ap()
    boutp = nc.dram_tensor("boutp", [1, 1], F32, kind="ExternalInput").ap()
    ones = nc.dram_tensor("ones", [1, W], BF16, kind="ExternalInput").ap()
    out = nc.dram_tensor("out", [S, BL], F32, kind="ExternalOutput").ap()
    out_flat = out.rearrange("s b -> (s b)")

    with tile.TileContext(nc) as tc:
        from contextlib import ExitStack

        stack = ExitStack()
        consts = stack.enter_context(tc.tile_pool(name="consts", bufs=1))

        # ---- persistent SBUF constants (all bf16 matmul operands) ----
        whh_sb = consts.tile([H, 12 * H], BF16)  # (l,d,g) blocks of 128 cols
        for k in range(12):
            nc.sync.dma_start(whh_sb[:, k * H:(k + 1) * H], whhT[k])
        wih0_sb = consts.tile([DIN + 1, 2 * 3 * H], BF16)
        for d in range(2):
            nc.sync.dma_start(wih0_sb[:, d * 3 * H:(d + 1) * 3 * H], wih0T[d])
        wih1_sb = consts.tile([H, 4 * 3 * H], BF16)  # (d,k) blocks of 384 cols
        for d in range(2):
            for k in range(2):
                c0 = (d * 2 + k) * 3 * H
                nc.sync.dma_start(wih1_sb[:, c0:c0 + 3 * H], wih1T[d, k])
        bias1_sb = consts.tile([1, 2 * 3 * H], BF16)
        nc.sync.dma_start(bias1_sb[:], bias1.rearrange("d m -> (d m)"))
        bhhn_sb = consts.tile([2, 2 * H], BF16)  # [dir_row, layer*128+col]
        for l in range(2):
            nc.sync.dma_start(bhhn_sb[:, l * H:(l + 1) * H], bhhn2[l])
        sel2_sb = consts.tile([2, 2 * BL], BF16)
        nc.sync.dma_start(sel2_sb[:], sel2[:])
        wout_sb = consts.tile([H, 2], BF16)
        nc.sync.dma_start(wout_sb[:], woutp[:])
        bout_sb = consts.tile([1, 1], F32)
        nc.sync.dma_start(bout_sb[:], boutp[:])
        ones_sb = consts.tile([1, W], BF16)
        nc.sync.dma_start(ones_sb[:], ones[:])
        hstate = consts.tile([H, 2 * BL], BF16)

        # ---- internal DRAM: layer outputs, padded one chunk on BOTH ends
        # (live rows [C, C+S)) so the i+1 forward and mirror prefetches stay
        # in bounds on the last iteration ----
        SP = S + 2 * C
        h0f = nc.dram_tensor("h0f", [H, SP, BL], BF16, kind="Internal").ap()
        h0b = nc.dram_tensor("h0b", [H, SP, BL], BF16, kind="Internal").ap()
        h1f = nc.dram_tensor("h1f", [H, SP, BL], BF16, kind="Internal").ap()
        h1b = nc.dram_tensor("h1b", [H, SP, BL], BF16, kind="Internal").ap()

        def whh(l, d, g):
            k = (l * 2 + d) * 3 + g
            return whh_sb[:, k * H:(k + 1) * H]

        rec = ExitStack()
        rhsp = rec.enter_context(tc.tile_pool(name="rhsp", bufs=1))
        ringp = rec.enter_context(tc.tile_pool(name="ringp", bufs=1))
        stepp = rec.enter_context(tc.tile_pool(name="stepp", bufs=3))
        psp = rec.enter_context(tc.tile_pool(name="psp", bufs=1, space="PSUM"))
        psnp = rec.enter_context(tc.tile_pool(name="psnp", bufs=2, space="PSUM"))

        # persistent per-layer tiles (no rotation inside the hardware loop)
        rz_h = [psp.tile([H, 4, W2], F32, tag=f"rz{h}", name=f"rz{h}")
                for h in range(2)]
        nj_h = [psp.tile([H, 2, W2], F32, tag=f"nj{h}", name=f"nj{h}")
                for h in range(2)]
        ring = ringp.tile([H, 2, C, BL], BF16, tag="ring", name="ring")
        xch = rhsp.tile([DIN + 1, W], BF16, tag="xch", name="xch")
        xrch = rhsp.tile([DIN + 1, W], BF16, tag="xrch", name="xrch")
        ff = rhsp.tile([H, C, BL], BF16, tag="ff", name="ff")
        bm = rhsp.tile([H, C, BL], BF16, tag="bm", name="bm")
        fm = rhsp.tile([H, C, BL], BF16, tag="fm", name="fm")
        bb = rhsp.tile([H, C, BL], BF16, tag="bb", name="bb")

        def emit_loads(l, idx):
            """DMAs for chunk `idx` into the persistent rhs tiles."""
            if l == 0:
                nc.sync.dma_start(xch[:], xf[:, ds(idx * W, W)])
                nc.sync.dma_start(xrch[:], xr[:, ds(idx * W, W)])
                return (xch, xrch)
            nc.sync.dma_start(ff[:], h0f[:][:, ds(idx * C + C, C), :])
            nc.sync.dma_start(bm[:], h0b[:][:, ds((NCH - idx) * C, C), :])
            nc.sync.dma_start(fm[:], h0f[:][:, ds((NCH - idx) * C, C), :])
            nc.sync.dma_start(bb[:], h0b[:][:, ds(idx * C + C, C), :])
            return (ff, bm, fm, bb)

        def rev_half(t, h):
            # steps [h*C2,(h+1)*C2) of the reversed view of t's C axis
            sl = (slice(C2 - 1, None, -1) if h == 1
                  else slice(C - 1, C2 - 1, -1))
            return t[:, sl, :]

        def emit_bulk(l, h, rhs):
            """Bulk gx matmuls for half h into rz_h[h] / nj_h[h]."""
            rz_ps, nj_ps = rz_h[h], nj_h[h]
            hs = slice(h * W2, (h + 1) * W2)
            seen_banks = set()

            def rz_start(sl):
                bank = sl // 2  # two 256-col fp32 slices per 2KB bank
                if bank in seen_banks:
                    return False
                seen_banks.add(bank)
                return True

            if l == 0:
                xch, xrch = rhs
                for dd, src in enumerate((xch, xrch)):
                    sh = src[:, hs]
                    for g in range(2):  # r, z
                        nc.tensor.matmul(
                            rz_ps[:, 2 * g + dd, :],
                            wih0_sb[:, dd * 3 * H + g * H: dd * 3 * H + (g + 1) * H],
                            sh, start=rz_start(2 * g + dd), stop=False,
                            skip_group_check=True)
                    nc.tensor.matmul(
                        nj_ps[:, dd, :],
                        wih0_sb[:, dd * 3 * H + 2 * H: dd * 3 * H + 3 * H],
                        sh, start=(dd == 0), stop=(dd == 1),
                        skip_group_check=True)
            else:
                ff, bm, fm, bb = rhs
                hsl = slice(h * C2, (h + 1) * C2)
                pairs = ((ff[:, hsl, :], rev_half(bm, h)),
                         (rev_half(fm, h), bb[:, hsl, :]))
                ones_h = ones_sb[:, 0:W2]
                for dd, (rA, rB) in enumerate(pairs):
                    base = dd * 2 * 3 * H
                    for g in range(2):
                        dst = rz_ps[:, 2 * g + dd, :]
                        nc.tensor.matmul(dst, wih1_sb[:, base + g * H: base + (g + 1) * H],
                                         rA, start=rz_start(2 * g + dd), stop=False,
                                         skip_group_check=True)
                        nc.tensor.matmul(dst, wih1_sb[:, base + 3 * H + g * H: base + 3 * H + (g + 1) * H],
                                         rB, start=False, stop=False, skip_group_check=True)
                        nc.tensor.matmul(dst, bias1_sb[:, dd * 3 * H + g * H: dd * 3 * H + (g + 1) * H],
                                         ones_h, start=False, stop=False, skip_group_check=True)
                    dst = nj_ps[:, dd, :]
                    nc.tensor.matmul(dst, wih1_sb[:, base + 2 * H: base + 3 * H],
                                     rA, start=(dd == 0), stop=False, skip_group_check=True)
                    nc.tensor.matmul(dst, wih1_sb[:, base + 3 * H + 2 * H: base + 3 * H + 3 * H],
                                     rB, start=False, stop=False, skip_group_check=True)
                    nc.tensor.matmul(dst, bias1_sb[:, dd * 3 * H + 2 * H: dd * 3 * H + 3 * H],
                                     ones_h, start=False, stop=(dd == 1), skip_group_check=True)

        def emit_step(l, j):
            h = j // C2
            jj = j % C2
            rz_ps, nj_ps = rz_h[h], nj_h[h]
            if j == 0 or STEP_MODE == "nochain":
                hf, hb = hstate[:, 0:BL], hstate[:, BL:2 * BL]
            else:
                hf, hb = ring[:, 0, j - 1, :], ring[:, 1, j - 1, :]
            js = slice(jj * BL, (jj + 1) * BL)

            # r,z gates first (sigmoid waits only on these), accumulating
            # onto the prefilled gx slices
            nc.tensor.matmul(rz_ps[:, 0, js], whh(l, 0, 0), hf,
                             start=False, stop=False, skip_group_check=True)
            nc.tensor.matmul(rz_ps[:, 1, js], whh(l, 1, 0), hb,
                             start=False, stop=False, skip_group_check=True)
            nc.tensor.matmul(rz_ps[:, 2, js], whh(l, 0, 1), hf,
                             start=False, stop=False, skip_group_check=True)
            nc.tensor.matmul(rz_ps[:, 3, js], whh(l, 1, 1), hb,
                             start=False, stop=(jj == C2 - 1), skip_group_check=True)
            # hn = b_hh_n + W_hh_n @ h  (both dirs) in small psum
            psn = psnp.tile([H, 2 * BL], F32, tag="psn")
            nc.tensor.matmul(psn[:], bhhn_sb[:, l * H:(l + 1) * H], sel2_sb[:],
                             start=True, stop=False, skip_group_check=True)
            nc.tensor.matmul(psn[:, 0:BL], whh(l, 0, 2), hf,
                             start=False, stop=False, skip_group_check=True)
            nc.tensor.matmul(psn[:, BL:2 * BL], whh(l, 1, 2), hb,
                             start=False, stop=True, skip_group_check=True)
            # rz slices: 0:2 = r (f,b); 2:4 = zbar = 1-z (z weights negated on
            # host). Two ACTIVATEs so the r half fires off just the r matmuls.
            rz = stepp.tile([H, 4, BL], BF16, tag="rz")
            nc.scalar.activation(rz[:, 0:2, :], rz_ps[:, 0:2, js], AF.Sigmoid)
            nc.scalar.activation(rz[:, 2:4, :], rz_ps[:, 2:4, js], AF.Sigmoid)
            h_prev = (hstate[:, :] if (j == 0 or STEP_MODE == "nochain")
                      else ring[:, :, j - 1, :])
            rn = stepp.tile([H, 2 * BL], F32, tag="rn")
            nc.vector.tensor_mul(rn[:], rz[:, 0:2, :], psn[:])
            arg = stepp.tile([H, 2 * BL], F32, tag="arg")
            nc.vector.tensor_add(arg[:], rn[:], nj_ps[:, :, js])
            # nt2 = (zbar - 1)*h runs on DVE during the tanh;
            # h' = zbar*n + h - zbar*h = nz - nt2
            nt2 = stepp.tile([H, 2 * BL], F32, tag="nt2")
            nc.vector.scalar_tensor_tensor(
                nt2[:], rz[:, 2:4, :], 1.0, h_prev,
                op0=ALU.subtract, op1=ALU.mult)
            n_t = stepp.tile([H, 2 * BL], BF16, tag="n")
            nc.scalar.activation(n_t[:], arg[:], AF.Tanh)
            nz = stepp.tile([H, 2 * BL], F32, tag="nz")
            nc.vector.tensor_mul(nz[:], rz[:, 2:4, :], n_t[:])
            nc.vector.tensor_sub(ring[:, :, j, :], nz[:], nt2[:])

        NBLK = 8  # ring->DRAM streaming granularity (steps per block)
        BSTEP = C // NBLK

        def emit_layer(l):
            nc.vector.memset(hstate[:], 0.0)
            h_f_dst, h_b_dst = (h0f, h0b) if l == 0 else (h1f, h1b)
            hints = (mybir.EngineType.PE, mybir.EngineType.DVE) if USE_HINTS else ()
            # prologue: chunk 0 loads + A-half bulk
            rhs = emit_loads(l, 0)
            emit_bulk(l, 0, rhs)
            with tc.For_i(0, NCH, 1, name=f"layer{l}", hint_engines=hints,
                          staggered_reset=True) as i:
                emit_bulk(l, 1, rhs)     # B half of chunk i
                emit_loads(l, i + 1)     # prefetch chunk i+1 (same tiles)
                for j in range(C):
                    emit_step(l, j)
                    if j % BSTEP == BSTEP - 1:
                        blk = ds(i * C + C + (j + 1 - BSTEP), BSTEP)
                        bs = slice(j + 1 - BSTEP, j + 1)
                        nc.sync.dma_start(h_f_dst[:][:, blk, :], ring[:, 0, bs, :])
                        nc.sync.dma_start(h_b_dst[:][:, blk, :], ring[:, 1, bs, :])
                    if j == C2 - 1:
                        emit_bulk(l, 0, rhs)  # A half of chunk i+1
                nc.vector.tensor_copy(hstate[:], ring[:, :, C - 1, :])

        emit_layer(0)
        emit_layer(1)
        rec.close()

        # ---- head: logits = wout_f . f1[s] + wout_b . b1[s] + bout ----
        with tc.tile_pool(name="headp", bufs=3) as hp, \
             tc.tile_pool(name="headps", bufs=2, space="PSUM") as hps:
            for k in range(NCH):
                fch = hp.tile([H, W], BF16, tag="fch")
                nc.sync.dma_start(fch[:], h1f[:][:, C + k * C: C + (k + 1) * C, :])
                bch = hp.tile([H, C, BL], BF16, tag="bch")
                mk = NCH - 1 - k
                nc.sync.dma_start(bch[:], h1b[:][:, C + mk * C: C + (mk + 1) * C, :])
                pso = hps.tile([1, W], F32, tag="pso")
                nc.tensor.matmul(pso[:], wout_sb[:, 0:1], fch[:],
                                 start=True, stop=False, skip_group_check=True)
                nc.tensor.matmul(pso[:], wout_sb[:, 1:2], bch[:, ::-1, :],
                                 start=False, stop=True, skip_group_check=True)
                osb = hp.tile([1, W], F32, tag="osb")
                nc.scalar.activation(osb[:], pso[:], AF.Identity,
                                     bias=bout_sb[0:1, 0:1])
                nc.sync.dma_start(out_flat[k * W:(k + 1) * W], osb[:])
        stack.close()

    nc.compile()
    return nc


_PROGRAM_CACHE = {}


def _get_program(S=4096, C=64):
    key = (S, C)
    if key not in _PROGRAM_CACHE:
        _PROGRAM_CACHE[key] = build_program(S, C)
    return _PROGRAM_CACHE[key]


def _pack_host_inputs(inputs, S=4096, C=64):
    """Build the per-core input maps from the full problem inputs."""
    W = C * BL
    x = np.asarray(inputs["x"], np.float32)

    # z-gate (g==1) weights and biases are negated so the kernel's sigmoid
    # yields zbar = 1 - z directly (h' = zbar*n + (h - zbar*h)).
    def gsign(g):
        return -1.0 if g == 1 else 1.0

    def gT(w, g):  # transposed gate block: [in, H]
        return np.ascontiguousarray(
            gsign(g) * np.asarray(w, np.float32)[g * H:(g + 1) * H].T)

    whhT = np.stack([
        gT(inputs[f"whh{l}{d}"], g)
        for l in range(2) for d in "fb" for g in range(3)
    ])  # [12,H,H]

    wih0T = np.zeros((2, DIN + 1, 3 * H), np.float32)
    bhhn2 = np.zeros((2, 2, H), np.float32)
    for di, d in enumerate("fb"):
        wih = np.asarray(inputs[f"wih0{d}"], np.float32)  # [3H, DIN]
        bih = np.asarray(inputs[f"bih0{d}"], np.float32)
        bhh = np.asarray(inputs[f"bhh0{d}"], np.float32)
        wih0T[di, :DIN] = wih.T
        for g in range(3):
            bias = bih[g * H:(g + 1) * H].copy()
            if g < 2:
                bias += bhh[g * H:(g + 1) * H]
            wih0T[di, DIN, g * H:(g + 1) * H] = gsign(g) * bias
        wih0T[di, :DIN, H:2 * H] *= -1.0
        bhhn2[0, di] = bhh[2 * H:]

    wih1T = np.zeros((2, 2, H, 3 * H), np.float32)
    bias1 = np.zeros((2, 3 * H), np.float32)
    for di, d in enumerate("fb"):
        wih = np.asarray(inputs[f"wih1{d}"], np.float32)  # [3H, 2H]
        bih = np.asarray(inputs[f"bih1{d}"], np.float32)
        bhh = np.asarray(inputs[f"bhh1{d}"], np.float32)
        for k in range(2):
            for g in range(3):
                wih1T[di, k, :, g * H:(g + 1) * H] = (
                    gsign(g) * wih[g * H:(g + 1) * H, k * H:(k + 1) * H].T)
        for g in range(3):
            bias = bih[g * H:(g + 1) * H].copy()
            if g < 2:
                bias += bhh[g * H:(g + 1) * H]
            bias1[di, g * H:(g + 1) * H] = gsign(g) * bias
        bhhn2[1, di] = bhh[2 * H:]

    sel2 = np.zeros((2, 2 * BL), np.float32)
    sel2[0, :BL] = 1.0
    sel2[1, BL:] = 1.0
    woutp = np.zeros((H, 2), np.float32)
    wout = np.asarray(inputs["wout"], np.float32)
    woutp[:, 0] = wout[0, :H]
    woutp[:, 1] = wout[0, H:]
    boutp = np.asarray(inputs["bout"], np.float32).reshape(1, 1)
    ones = np.ones((1, W), np.float32)

    bf = lambda a: np.ascontiguousarray(a.astype(NP_BF16))
    shared = dict(whhT=bf(whhT), wih0T=bf(wih0T), wih1T=bf(wih1T),
                  bias1=bf(bias1), bhhn2=bf(bhhn2), sel2=bf(sel2),
                  woutp=bf(woutp), boutp=boutp, ones=bf(ones))

    in_maps = []
    for c in range(NCORES):
        xc = x[c * BL:(c + 1) * BL]  # [BL, S, DIN]
        arr = np.ones((DIN + 1, S + C, BL), np.float32)
        arr[:DIN, :S] = xc.transpose(2, 1, 0)
        arr[:DIN, S:] = 0.0
        xfm = bf(arr.reshape(DIN + 1, (S + C) * BL))
        arrr = arr.copy()
        arrr[:, :S] = arr[:, :S][:, ::-1]
        xrm = bf(np.ascontiguousarray(arrr.reshape(DIN + 1, (S + C) * BL)))
        in_maps.append(dict(shared, xf=xfm, xr=xrm))
    return in_maps


def kernel(**inputs) -> np.ndarray:
    from concourse import bass_utils
    S, C = 4096, 64
    nc = _get_program(S, C)
    in_maps = _pack_host_inputs(inputs, S, C)
    res = bass_utils.run_bass_kernel_spmd(nc, in_maps, core_ids=list(range(NCORES)))
    outs = [r["out"] for r in res.results]  # each [S, BL]
    return np.concatenate([o.T for o in outs], axis=0).astype(np.float32)


# revision 24
# speedup vs baseline: 2.5426x; 1.0025x over previous
"""Trainium2 Bass kernel for a 2-layer bidirectional GRU + linear head.

Problem: B=64, S=4096, D_IN=7, H=128, PyTorch gate order (r, z, n).
Sharding: data-parallel over batch across 8 NeuronCores (8 rows each).

Per-core design (all layouts keep H=128 on the SBUF partition axis):
  - The sequence is processed in chunks of C=64 steps, each split into two
    32-step halves (A/B) with separate PSUM tiles. The input-gate projections
    gx = W_ih @ x (+ biases) for r,z go into 2-bank PSUM tiles per half; the
    per-step recurrent matmuls (W_hh @ h) accumulate onto their 8-column
    slice (start=False), so sigmoid reads (xr+hr, xz+hz) straight out of
    PSUM. The n-gate projections live in their own 1-bank PSUM tile per half
    and are read directly by the DVE (no SBUF staging).
  - Software pipelining across chunks: chunk i+1's input DMAs issue before
    chunk i's steps run (transfers overlap compute), and chunk i+1's A-half
    bulk matmuls are emitted after chunk i's A-half steps, so they execute
    mid-chunk (their PSUM WAR is against step 31's sigmoid, not step 63's).
    DRAM tensors are padded one chunk so the i+1 prefetch never reads OOB.
  - Both directions are packed into the free dim of every elementwise op
    (columns 0:8 forward, 8:16 backward); the backward direction consumes a
    host-reversed copy of x so all its tensors are in scan order, and the
    time-reversal for layer 1 / the head is applied via negative-stride SBUF
    views on the matmul rhs (contiguous DMAs; no per-row descriptors).
  - The hidden-state ring [128, 2, C, BL] doubles as the output buffer and
    is streamed to DRAM in 8-step blocks as the h' writes land, so the next
    chunk's step-0 write never waits on a whole-ring DMA.
  - The GRU update uses zbar = 1-z (z weights negated on host):
    h' = zbar*n - ((zbar-1)*h), with (zbar-1)*h computed by a fused
    scalar_tensor_tensor on the DVE during the tanh.
  - All matmul operands are bf16 (PSUM accumulation and every elementwise op
    stay fp32): fp32 matmuls are decomposed by HW into TWO ldweights+matmul
    passes and disable Fast Weight Load, roughly doubling PE occupancy, and
    fp32 PE activity triggers the HAM power throttle (50% duty cycle).
"""

import numpy as np
import ml_dtypes

import concourse.bass as bass
import concourse.tile as tile
from concourse import bacc, mybir
from concourse.bass import ds

F32 = mybir.dt.float32
BF16 = mybir.dt.bfloat16
NP_BF16 = ml_dtypes.bfloat16
AF = mybir.ActivationFunctionType
ALU = mybir.AluOpType

H = 128
DIN = 7
B = 64
NCORES = 8
BL = B // NCORES  # batch rows per core

STEP_MODE = "full"   # "full" | "nochain" (steps read hstate, no serial dep)
USE_HINTS = True


def build_program(S=4096, C=64, n_cores=NCORES):
    """Build the per-core Bass program."""
    NCH = S // C
    W = C * BL       # chunk columns, 512 for C=64
    C2 = C // 2      # steps per half-chunk
    W2 = C2 * BL     # half-chunk columns, 256
    nc = bacc.Bacc("TRN2", target_bir_lowering=False, debug=False)

    # ---- DRAM I/O (x padded one chunk at the end for the i+1 prefetch) ----
    xf = nc.dram_tensor("xf", [DIN + 1, (S + C) * BL], BF16, kind="ExternalInput").ap()
    xr = nc.dram_tensor("xr", [DIN + 1, (S + C) * BL], BF16, kind="ExternalInput").ap()
    whhT = nc.dram_tensor("whhT", [12, H, H], BF16, kind="ExternalInput").ap()
    wih0T = nc.dram_tensor("wih0T", [2, DIN + 1, 3 * H], BF16, kind="ExternalInput").ap()
    wih1T = nc.dram_tensor("wih1T", [2, 2, H, 3 * H], BF16, kind="ExternalInput").ap()
    bias1 = nc.dram_tensor("bias1", [2, 3 * H], BF16, kind="ExternalInput").ap()
    bhhn2 = nc.dram_tensor("bhhn2", [2, 2, H], BF16, kind="ExternalInput").ap()
    sel2 = nc.dram_tensor("sel2", [2, 2 * BL], BF16, kind="ExternalInput").ap()
    woutp = nc.dram_tensor("woutp", [H, 2], BF16, kind="ExternalInput").ap()
    boutp = nc.dram_tensor("boutp", [1, 1], F32, kind="ExternalInput").ap()
    ones = nc.dram_tensor("ones", [1, W], BF16, kind="ExternalInput").ap()
    out = nc.dram_tensor("out", [S, BL], F32, kind="ExternalOutput").ap()
    out_flat = out.rearrange("s b -> (s b)")

    with tile.TileContext(nc) as tc:
        from contextlib import ExitStack

        stack = ExitStack()
        consts = stack.enter_context(tc.tile_pool(name="consts", bufs=1))

        # ---- persistent SBUF constants (all bf16 matmul operands) ----
        whh_sb = consts.tile([H, 12 * H], BF16)  # (l,d,g) blocks of 128 cols
        for k in range(12):
            nc.sync.dma_start(whh_sb[:, k * H:(k + 1) * H], whhT[k])
        wih0_sb = consts.tile([DIN + 1, 2 * 3 * H], BF16)
        for d in range(2):
            nc.sync.dma_start(wih0_sb[:, d * 3 * H:(d + 1) * 3 * H], wih0T[d])
        wih1_sb = consts.tile([H, 4 * 3 * H], BF16)  # (d,k) blocks of 384 cols
        for d in range(2):
            for k in range(2):
                c0 = (d * 2 + k) * 3 * H
                nc.sync.dma_start(wih1_sb[:, c0:c0 + 3 * H], wih1T[d, k])
        bias1_sb = consts.tile([1, 2 * 3 * H], BF16)
        nc.sync.dma_start(bias1_sb[:], bias1.rearrange("d m -> (d m)"))
        bhhn_sb = consts.tile([2, 2 * H], BF16)  # [dir_row, layer*128+col]
        for l in range(2):
            nc.sync.dma_start(bhhn_sb[:, l * H:(l + 1) * H], bhhn2[l])
        sel2_sb = consts.tile([2, 2 * BL], BF16)
        nc.sync.dma_start(sel2_sb[:], sel2[:])
        wout_sb = consts.tile([H, 2], BF16)
        nc.sync.dma_start(wout_sb[:], woutp[:])
        bout_sb = consts.tile([1, 1], F32)
        nc.sync.dma_start(bout_sb[:], boutp[:])
        ones_sb = consts.tile([1, W], BF16)
        nc.sync.dma_start(ones_sb[:], ones[:])
        hstate = consts.tile([H, 2 * BL], BF16)

        # ---- internal DRAM: layer outputs, padded one chunk on BOTH ends
        # (live rows [C, C+S)) so the i+1 forward and mirror prefetches stay
        # in bounds on the last iteration ----
        SP = S + 2 * C
        h0f = nc.dram_tensor("h0f", [H, SP, BL], BF16, kind="Internal").ap()
        h0b = nc.dram_tensor("h0b", [H, SP, BL], BF16, kind="Internal").ap()
        h1f = nc.dram_tensor("h1f", [H, SP, BL], BF16, kind="Internal").ap()
        h1b = nc.dram_tensor("h1b", [H, SP, BL], BF16, kind="Internal").ap()

        def whh(l, d, g):
            k = (l * 2 + d) * 3 + g
            return whh_sb[:, k * H:(k + 1) * H]

        rec = ExitStack()
        rhsp = rec.enter_context(tc.tile_pool(name="rhsp", bufs=1))
        ringp = rec.enter_context(tc.tile_pool(name="ringp", bufs=1))
        stepp = rec.enter_context(tc.tile_pool(name="stepp", bufs=3))
        psp = rec.enter_context(tc.tile_pool(name="psp", bufs=1, space="PSUM"))
        psnp = rec.enter_context(tc.tile_pool(name="psnp", bufs=2, space="PSUM"))

        # persistent per-layer tiles (no rotation inside the hardware loop)
        rz_h = [psp.tile([H, 4, W2], F32, tag=f"rz{h}", name=f"rz{h}")
                for h in range(2)]
        nj_h = [psp.tile([H, 2, W2], F32, tag=f"nj{h}", name=f"nj{h}")
                for h in range(2)]
        ring = ringp.tile([H, 2, C, BL], BF16, tag="ring", name="ring")
        xch = rhsp.tile([DIN + 1, W], BF16, tag="xch", name="xch")
        xrch = rhsp.tile([DIN + 1, W], BF16, tag="xrch", name="xrch")
        ff = rhsp.tile([H, C, BL], BF16, tag="ff", name="ff")
        bm = rhsp.tile([H, C, BL], BF16, tag="bm", name="bm")
        fm = rhsp.tile([H, C, BL], BF16, tag="fm", name="fm")
        bb = rhsp.tile([H, C, BL], BF16, tag="bb", name="bb")

        def emit_loads(l, idx):
            """DMAs for chunk `idx` into the persistent rhs tiles."""
            if l == 0:
                nc.sync.dma_start(xch[:], xf[:, ds(idx * W, W)])
                nc.sync.dma_start(xrch[:], xr[:, ds(idx * W, W)])
                return (xch, xrch)
            nc.sync.dma_start(ff[:], h0f[:][:, ds(idx * C + C, C), :])
            nc.sync.dma_start(bm[:], h0b[:][:, ds((NCH - idx) * C, C), :])
            nc.sync.dma_start(fm[:], h0f[:][:, ds((NCH - idx) * C, C), :])
            nc.sync.dma_start(bb[:], h0b[:][:, ds(idx * C + C, C), :])
            return (ff, bm, fm, bb)

        def rev_half(t, h):
            # steps [h*C2,(h+1)*C2) of the reversed view of t's C axis
            sl = (slice(C2 - 1, None, -1) if h == 1
                  else slice(C - 1, C2 - 1, -1))
            return t[:, sl, :]

        def emit_bulk(l, h, rhs):
            """Bulk gx matmuls for half h into rz_h[h] / nj_h[h]."""
            rz_ps, nj_ps = rz_h[h], nj_h[h]
            hs = slice(h * W2, (h + 1) * W2)
            seen_banks = set()

            def rz_start(sl):
                bank = sl // 2  # two 256-col fp32 slices per 2KB bank
                if bank in seen_banks:
                    return False
                seen_banks.add(bank)
                return True

            if l == 0:
                xch, xrch = rhs
                for dd, src in enumerate((xch, xrch)):
                    sh = src[:, hs]
                    for g in range(2):  # r, z
                        nc.tensor.matmul(
                            rz_ps[:, 2 * g + dd, :],
                            wih0_sb[:, dd * 3 * H + g * H: dd * 3 * H + (g + 1) * H],
                            sh, start=rz_start(2 * g + dd), stop=False,
                            skip_group_check=True)
                    nc.tensor.matmul(
                        nj_ps[:, dd, :],
                        wih0_sb[:, dd * 3 * H + 2 * H: dd * 3 * H + 3 * H],
                        sh, start=(dd == 0), stop=(dd == 1),
                        skip_group_check=True)
            else:
                ff, bm, fm, bb = rhs
                hsl = slice(h * C2, (h + 1) * C2)
                pairs = ((ff[:, hsl, :], rev_half(bm, h)),
                         (rev_half(fm, h), bb[:, hsl, :]))
                ones_h = ones_sb[:, 0:W2]
                for dd, (rA, rB) in enumerate(pairs):
                    base = dd * 2 * 3 * H
                    for g in range(2):
                        dst = rz_ps[:, 2 * g + dd, :]
                        nc.tensor.matmul(dst, wih1_sb[:, base + g * H: base + (g + 1) * H],
                                         rA, start=rz_start(2 * g + dd), stop=False,
                                         skip_group_check=True)
                        nc.tensor.matmul(dst, wih1_sb[:, base + 3 * H + g * H: base + 3 * H + (g + 1) * H],
                                         rB, start=False, stop=False, skip_group_check=True)
                        nc.tensor.matmul(dst, bias1_sb[:, dd * 3 * H + g * H: dd * 3 * H + (g + 1) * H],
                                         ones_h, start=False, stop=False, skip_group_check=True)
                    dst = nj_ps[:, dd, :]
                    nc.tensor.matmul(dst, wih1_sb[:, base + 2 * H: base + 3 * H],
                                     rA, start=(dd == 0), stop=False, skip_group_check=True)
                    nc.tensor.matmul(dst, wih1_sb[:, base + 3 * H + 2 * H: base + 3 * H + 3 * H],
                                     rB, start=False, stop=False, skip_group_check=True)
                    nc.tensor.matmul(dst, bias1_sb[:, dd * 3 * H + 2 * H: dd * 3 * H + 3 * H],
                                     ones_h, start=False, stop=(dd == 1), skip_group_check=True)

        def emit_step(l, j):
            h = j // C2
            jj = j % C2
            rz_ps, nj_ps = rz_h[h], nj_h[h]
            if j == 0 or STEP_MODE == "nochain":
                hf, hb = hstate[:, 0:BL], hstate[:, BL:2 * BL]
            else:
                hf, hb = ring[:, 0, j - 1, :], ring[:, 1, j - 1, :]
            js = slice(jj * BL, (jj + 1) * BL)

            # r,z gates first (sigmoid waits only on these), accumulating
            # onto the prefilled gx slices
            nc.tensor.matmul(rz_ps[:, 0, js], whh(l, 0, 0), hf,
                             start=False, stop=False, skip_group_check=True)
            nc.tensor.matmul(rz_ps[:, 1, js], whh(l, 1, 0), hb,
                             start=False, stop=False, skip_group_check=True)
            nc.tensor.matmul(rz_ps[:, 2, js], whh(l, 0, 1), hf,
                             start=False, stop=False, skip_group_check=True)
            nc.tensor.matmul(rz_ps[:, 3, js], whh(l, 1, 1), hb,
                             start=False, stop=(jj == C2 - 1), skip_group_check=True)
            # hn = b_hh_n + W_hh_n @ h  (both dirs) in small psum
            psn = psnp.tile([H, 2 * BL], F32, tag="psn")
            nc.tensor.matmul(psn[:], bhhn_sb[:, l * H:(l + 1) * H], sel2_sb[:],
                             start=True, stop=False, skip_group_check=True)
            nc.tensor.matmul(psn[:, 0:BL], whh(l, 0, 2), hf,
                             start=False, stop=False, skip_group_check=True)
            nc.tensor.matmul(psn[:, BL:2 * BL], whh(l, 1, 2), hb,
                             start=False, stop=True, skip_group_check=True)
            # rz slices: 0:2 = r (f,b); 2:4 = zbar = 1-z (z weights negated on
            # host). Two ACTIVATEs so the r half fires off just the r matmuls.
            rz = stepp.tile([H, 4, BL], BF16, tag="rz")
            nc.scalar.activation(rz[:, 0:2, :], rz_ps[:, 0:2, js], AF.Sigmoid)
            nc.scalar.activation(rz[:, 2:4, :], rz_ps[:, 2:4, js], AF.Sigmoid)
            h_prev = (hstate[:, :] if (j == 0 or STEP_MODE == "nochain")
                      else ring[:, :, j - 1, :])
            rn = stepp.tile([H, 2 * BL], F32, tag="rn")
            nc.vector.tensor_mul(rn[:], rz[:, 0:2, :], psn[:])
            arg = stepp.tile([H, 2 * BL], F32, tag="arg")
            nc.vector.tensor_add(arg[:], rn[:], nj_ps[:, :, js])
            # nt2 = (zbar - 1)*h runs on DVE during the tanh;
            # h' = zbar*n + h - zbar*h = nz - nt2
            nt2 = stepp.tile([H, 2 * BL], F32, tag="nt2")
            nc.vector.scalar_tensor_tensor(
                nt2[:], rz[:, 2:4, :], 1.0, h_prev,
                op0=ALU.subtract, op1=ALU.mult)
            n_t = stepp.tile([H, 2 * BL], BF16, tag="n")
            nc.scalar.activation(n_t[:], arg[:], AF.Tanh)
            nz = stepp.tile([H, 2 * BL], F32, tag="nz")
            nc.vector.tensor_mul(nz[:], rz[:, 2:4, :], n_t[:])
            nc.vector.tensor_sub(ring[:, :, j, :], nz[:], nt2[:])

        NBLK = 8  # ring->DRAM streaming granularity (steps per block)
        BSTEP = C // NBLK

        def emit_layer(l):
            nc.vector.memset(hstate[:], 0.0)
            nc.vector.memset(ring[:], 0.0)  # the i=0 deferred ring DMA reads it
            h_f_dst, h_b_dst = (h0f, h0b) if l == 0 else (h1f, h1b)
            hints = (mybir.EngineType.PE, mybir.EngineType.DVE) if USE_HINTS else ()
            # prologue: chunk 0 loads + both bulk halves
            rhs = emit_loads(l, 0)
            emit_bulk(l, 0, rhs)
            emit_bulk(l, 1, rhs)
            bs_last = slice(C - BSTEP, C)
            with tc.For_i(0, NCH, 1, name=f"layer{l}", hint_engines=hints,
                          staggered_reset=True) as i:
                # chunk i-1's deferred last ring block (pad garbage at i=0):
                # keeps its DMA completion off the loop's reset gate
                blkp = ds(i * C + C - BSTEP, BSTEP)
                nc.sync.dma_start(h_f_dst[:][:, blkp, :], ring[:, 0, bs_last, :])
                nc.sync.dma_start(h_b_dst[:][:, blkp, :], ring[:, 1, bs_last, :])
                emit_loads(l, i + 1)     # prefetch chunk i+1 (same tiles)
                for j in range(C):
                    emit_step(l, j)
                    if j % BSTEP == BSTEP - 1 and j != C - 1:
                        blk = ds(i * C + C + (j + 1 - BSTEP), BSTEP)
                        bs = slice(j + 1 - BSTEP, j + 1)
                        nc.sync.dma_start(h_f_dst[:][:, blk, :], ring[:, 0, bs, :])
                        nc.sync.dma_start(h_b_dst[:][:, blk, :], ring[:, 1, bs, :])
                    if j == C2 - 1:
                        emit_bulk(l, 0, rhs)  # A half of chunk i+1
                nc.vector.tensor_copy(hstate[:], ring[:, :, C - 1, :])
                emit_bulk(l, 1, rhs)     # B half of chunk i+1 (runs at boundary)
            # epilogue: last chunk's final ring block
            blkp = ds(NCH * C + C - BSTEP, BSTEP)
            nc.sync.dma_start(h_f_dst[:][:, blkp, :], ring[:, 0, bs_last, :])
            nc.sync.dma_start(h_b_dst[:][:, blkp, :], ring[:, 1, bs_last, :])

        emit_layer(0)
        emit_layer(1)
        rec.close()

        # ---- head: logits = wout_f . f1[s] + wout_b . b1[s] + bout ----
        with tc.tile_pool(name="headp", bufs=3) as hp, \
             tc.tile_pool(name="headps", bufs=2, space="PSUM") as hps:
            for k in range(NCH):
                fch = hp.tile([H, W], BF16, tag="fch")
                nc.sync.dma_start(fch[:], h1f[:][:, C + k * C: C + (k + 1) * C, :])
                bch = hp.tile([H, C, BL], BF16, tag="bch")
                mk = NCH - 1 - k
                nc.sync.dma_start(bch[:], h1b[:][:, C + mk * C: C + (mk + 1) * C, :])
                pso = hps.tile([1, W], F32, tag="pso")
                nc.tensor.matmul(pso[:], wout_sb[:, 0:1], fch[:],
                                 start=True, stop=False, skip_group_check=True)
                nc.tensor.matmul(pso[:], wout_sb[:, 1:2], bch[:, ::-1, :],
                                 start=False, stop=True, skip_group_check=True)
                osb = hp.tile([1, W], F32, tag="osb")
                nc.scalar.activation(osb[:], pso[:], AF.Identity,
                                     bias=bout_sb[0:1, 0:1])
                nc.sync.dma_start(out_flat[k * W:(k + 1) * W], osb[:])
        stack.close()

    nc.compile()
    return nc


_PROGRAM_CACHE = {}


def _get_program(S=4096, C=64):
    key = (S, C)
    if key not in _PROGRAM_CACHE:
        _PROGRAM_CACHE[key] = build_program(S, C)
    return _PROGRAM_CACHE[key]


def _pack_host_inputs(inputs, S=4096, C=64):
    """Build the per-core input maps from the full problem inputs."""
    W = C * BL
    x = np.asarray(inputs["x"], np.float32)

    # z-gate (g==1) weights and biases are negated so the kernel's sigmoid
    # yields zbar = 1 - z directly (h' = zbar*n + (h - zbar*h)).
    def gsign(g):
        return -1.0 if g == 1 else 1.0

    def gT(w, g):  # transposed gate block: [in, H]
        return np.ascontiguousarray(
            gsign(g) * np.asarray(w, np.float32)[g * H:(g + 1) * H].T)

    whhT = np.stack([
        gT(inputs[f"whh{l}{d}"], g)
        for l in range(2) for d in "fb" for g in range(3)
    ])  # [12,H,H]

    wih0T = np.zeros((2, DIN + 1, 3 * H), np.float32)
    bhhn2 = np.zeros((2, 2, H), np.float32)
    for di, d in enumerate("fb"):
        wih = np.asarray(inputs[f"wih0{d}"], np.float32)  # [3H, DIN]
        bih = np.asarray(inputs[f"bih0{d}"], np.float32)
        bhh = np.asarray(inputs[f"bhh0{d}"], np.float32)
        wih0T[di, :DIN] = wih.T
        for g in range(3):
            bias = bih[g * H:(g + 1) * H].copy()
            if g < 2:
                bias += bhh[g * H:(g + 1) * H]
            wih0T[di, DIN, g * H:(g + 1) * H] = gsign(g) * bias
        wih0T[di, :DIN, H:2 * H] *= -1.0
        bhhn2[0, di] = bhh[2 * H:]

    wih1T = np.zeros((2, 2, H, 3 * H), np.float32)
    bias1 = np.zeros((2, 3 * H), np.float32)
    for di, d in enumerate("fb"):
        wih = np.asarray(inputs[f"wih1{d}"], np.float32)  # [3H, 2H]
        bih = np.asarray(inputs[f"bih1{d}"], np.float32)
        bhh = np.asarray(inputs[f"bhh1{d}"], np.float32)
        for k in range(2):
            for g in range(3):
                wih1T[di, k, :, g * H:(g + 1) * H] = (
                    gsign(g) * wih[g * H:(g + 1) * H, k * H:(k + 1) * H].T)
        for g in range(3):
            bias = bih[g * H:(g + 1) * H].copy()
            if g < 2:
                bias += bhh[g * H:(g + 1) * H]
            bias1[di, g * H:(g + 1) * H] = gsign(g) * bias
        bhhn2[1, di] = bhh[2 * H:]

    sel2 = np.zeros((2, 2 * BL), np.float32)
    sel2[0, :BL] = 1.0
    sel2[1, BL:] = 1.0
    woutp = np.zeros((H, 2), np.float32)
    wout = np.asarray(inputs["wout"], np.float32)
    woutp[:, 0] = wout[0, :H]
    woutp[:, 1] = wout[0, H:]
    boutp = np.asarray(inputs["bout"], np.float32).reshape(1, 1)
    ones = np.ones((1, W), np.float32)

    bf = lambda a: np.ascontiguousarray(a.astype(NP_BF16))
    shared = dict(whhT=bf(whhT), wih0T=bf(wih0T), wih1T=bf(wih1T),
                  bias1=bf(bias1), bhhn2=bf(bhhn2), sel2=bf(sel2),
                  woutp=bf(woutp), boutp=boutp, ones=bf(ones))

    in_maps = []
    for c in range(NCORES):
        xc = x[c * BL:(c + 1) * BL]  # [BL, S, DIN]
        arr = np.ones((DIN + 1, S + C, BL), np.float32)
        arr[:DIN, :S] = xc.transpose(2, 1, 0)
        arr[:DIN, S:] = 0.0
        xfm = bf(arr.reshape(DIN + 1, (S + C) * BL))
        arrr = arr.copy()
        arrr[:, :S] = arr[:, :S][:, ::-1]
        xrm = bf(np.ascontiguousarray(arrr.reshape(DIN + 1, (S + C) * BL)))
        in_maps.append(dict(shared, xf=xfm, xr=xrm))
    return in_maps


def kernel(**inputs) -> np.ndarray:
    from concourse import bass_utils
    S, C = 4096, 64
    nc = _get_program(S, C)
    in_maps = _pack_host_inputs(inputs, S, C)
    res = bass_utils.run_bass_kernel_spmd(nc, in_maps, core_ids=list(range(NCORES)))
    outs = [r["out"] for r in res.results]  # each [S, BL]
    return np.concatenate([o.T for o in outs], axis=0).astype(np.float32)
